# revision 20
# baseline (speedup 1.0000x reference)
"""Causal single-head attention on 8 Trainium2 NeuronCores.

Problem: B=8, S=2048, E=768, HEAD=128, fp32.
  Xm = X * padding_mask[:, :, None]
  q/k/v = Xm @ W_{q,k,v}.T          [B, S, H]
  scores = (q @ k.T) / sqrt(H)  (causal)
  out = softmax(scores) @ v          [B, S, H]

Sharding: pure data-parallel over batch - core b computes batch b; the
tiny projection weights are replicated to every core.

v4 design notes (on top of the bf16 v2 baseline):
  - Startup: inputs stream in ordered globally by first use, with the
    critical sequence (w3[eo01], xt0 in 3 eo-pair chunks, then xt1
    chunks + the fp8 copies) on the sync queue, which serves ~1.5us
    after kernel start at ~250GB/s. Per-chunk completion semaphores let
    each projection matmul wait only on its own chunk. xt2/xt3 bulk is
    issued mid-attention from the scalar engine's program (engine-order
    staging) so it cannot steal startup bandwidth.
  - Projection matmuls for block b+1 are interleaved as small work
    items into the attention pair-pipeline of block b, filling the PE
    bubbles that previously appeared while waiting on ACT exp. Block
    3's projection is split: qT/kT items run inside attn(2), vT +
    transpose items inside attn(3)'s early pairs (legal: pair g only
    touches k-tiles 2g,2g+1, so block-3 k/v tiles are needed only from
    pair 6 on).
  - q/k projections for blocks 2-3 run as fp8(e4m3) DoubleRow matmuls
    (2 contraction elements per cycle, halving their PE time). X and W
    are pre-scaled (x4 / x64) on the host so W escapes the e4m3
    subnormal range; the x65536 score scale folds into the exp scale
    and a second (scaled) causal-mask constant. Host-simulated rel err
    is unchanged (4.8e-3) because the max-error rows live in blocks
    0-1, which stay bf16.
  - Per-block qT/kT/vT/v SBUF tiles so interleaved projection writes
    can never alias attention reads of earlier blocks.
  - outT is drained in bf16 (host divides in fp32); blocks 0-2 go out
    as single 1KB-line transfers on the otherwise-idle gpsimd queue,
    block 3 in two halves on scalar/sync as soon as each half's
    accumulation completes. den goes out as one 8KB DMA at the end.
"""

import math
import sys

import numpy as np

sys.path.insert(0, "/opt/trn_rl_repo")

import ml_dtypes

B, S, E, H = 8, 2048, 768, 128
EO = E // 128          # 6 e-chunks
NJB = S // 512         # 4 q-blocks of 512
SCALE = float(1.0 / math.sqrt(H))
XS, WS = 4.0, 64.0     # fp8 pre-scales for X and W_q/W_k
F8 = (2, 3)            # q-blocks whose q/k projections run in fp8

_CACHE = {}


def _emit_body(nc, tc, pools, dram):
    import concourse.bass as bass  # noqa: F401
    from concourse import mybir

    f32 = mybir.dt.float32
    bf16 = mybir.dt.bfloat16
    fp8 = mybir.dt.float8e4
    DR = mybir.MatmulPerfMode.DoubleRow
    Exp = mybir.ActivationFunctionType.Exp

    singles, prb_p, ps_proj, ps_sc, ps_o, ps_d = pools
    (xt_d, xt8_d, w3_d, w8_d, consts_d, outT_d, den_d) = dram

    sb = _CACHE["sb"]
    if not sb:
        for jb in range(NJB):
            sb[f"xt{jb}"] = singles.tile(
                [128, EO, 512], bf16, tag=f"xt{jb}", name=f"xt{jb}"
            )
            for nm in ("qT", "kT", "vT", "v"):
                sb[f"{nm}{jb}"] = singles.tile(
                    [128, 512], bf16, tag=f"{nm}{jb}", name=f"{nm}{jb}"
                )
            sb[f"outF{jb}"] = singles.tile(
                [128, 512], bf16, tag=f"outF{jb}", name=f"outF{jb}"
            )
        for jb in F8:
            sb[f"xt8_{jb}"] = singles.tile(
                [128, 3, 2, 512], fp8, tag=f"xt8_{jb}", name=f"xt8_{jb}"
            )
        sb["w8"] = singles.tile([128, 3, 2, 2, H], fp8, tag="w8", name="w8")
        sb["w3"] = singles.tile([128, EO, 3, H], bf16, tag="w3", name="w3")
        sb["consts"] = singles.tile([128, 4, 128], bf16, tag="consts", name="consts")
        sb["denF"] = singles.tile([1, S], f32, tag="denF", name="denF")
        sb["warm"] = singles.tile([128, 512], bf16, tag="warm", name="warm")

    xt_ap = xt_d.ap()
    xt8_ap = xt8_d.ap()
    w3_ap = w3_d.ap()
    outT_ap = outT_d.ap()
    den_ap = den_d.ap()

    # ---- prologue loads --------------------------------------------------
    # sync queue: the startup-critical sequence, ordered by first use.
    # scalar joins ~2us later with the rest of w3 + consts; its bulk
    # (xt2/xt3) is staged from inside attn(0)/attn(1) instead. gpsimd
    # stays free for output drains.
    nc.gpsimd.memset(sb["warm"], 0.125)
    nc.sync.dma_start(out=sb["w3"][:, 0:2], in_=w3_ap[:, 0:2])
    for p in range(6):
        nc.sync.dma_start(out=sb["xt0"][:, p : p + 1], in_=xt_ap[:, 0, p : p + 1])
    for p in range(3):
        nc.sync.dma_start(
            out=sb["xt1"][:, 2 * p : 2 * p + 2], in_=xt_ap[:, 1, 2 * p : 2 * p + 2]
        )
    nc.sync.dma_start(out=sb["w8"], in_=w8_d.ap())
    for i, jb in enumerate(F8):
        nc.sync.dma_start(out=sb[f"xt8_{jb}"], in_=xt8_ap[:, i])
    nc.scalar.dma_start(out=sb["w3"][:, 2:4], in_=w3_ap[:, 2:4])
    nc.scalar.dma_start(out=sb["w3"][:, 4:6], in_=w3_ap[:, 4:6])
    nc.scalar.dma_start(out=sb["consts"], in_=consts_d.ap())

    ident = sb["consts"][:, 0, :]
    triA = sb["consts"][:, 1, :]       # -400 (bf16-scale blocks)
    triA8 = sb["consts"][:, 2, :]      # -400 * (XS*WS)^2 (fp8-scale blocks)
    ones1 = sb["consts"][:, 3, 0:1]

    # PE warmup long enough (~3.4us of continuous activity) to flip the
    # HAM clock-gate to 2.4GHz BEFORE the first data-dependent matmul,
    # instead of hoping the chunk-paced projection stream does it later.
    ps_warm = ps_proj.tile([128, 512], f32, tag="proj", name="ps_warm")

    def warm_mm():
        nc.tensor.matmul(
            ps_warm, lhsT=sb["warm"][:, 0:128], rhs=sb["warm"], start=True, stop=True
        )

    for _ in range(9):
        warm_mm()

    # ---- projection work items ------------------------------------------
    # proj(b) = 11 small PE items: 3 per weight (chunk-paced; the last
    # one adds the PSUM->SBUF copy) + 2 transpose items for v. For F8
    # blocks the q/k items are single fp8 DoubleRow matmuls over an
    # eo-pair (256-wide contraction at 2 elems/cycle).
    def proj_items(b, parts=(0, 1, 2), chunk_major=False):
        items = []
        cell = {}
        use8 = b in F8
        x8 = sb[f"xt8_{b}"] if use8 else None

        def mm_item(wi, p, nm, b=b):
            def run():
                if p == 0:
                    cell[wi] = ps_proj.tile(
                        [128, 512], f32, tag="proj", name=f"ps_{nm}{b}"
                    )
                ps = cell[wi]
                if use8 and wi < 2:
                    nc.tensor.matmul(
                        ps,
                        lhsT=sb["w8"][:, p, :, wi, :],
                        rhs=x8[:, p, :, :],
                        start=(p == 0),
                        stop=(p == 2),
                        perf_mode=DR,
                    )
                else:
                    for eo in (2 * p, 2 * p + 1):
                        nc.tensor.matmul(
                            ps,
                            lhsT=sb["w3"][:, eo, wi, :],
                            rhs=sb[f"xt{b}"][:, eo, :],
                            start=(eo == 0),
                            stop=(eo == EO - 1),
                        )
                if p == 2:
                    nc.vector.tensor_copy(sb[f"{nm}{b}"], ps)

            return run

        def tr_item(half, b=b):
            def run():
                if half == 0:
                    cell["psv"] = ps_proj.tile(
                        [128, 512], bf16, tag="proj", name=f"psv{b}"
                    )
                psv = cell["psv"]
                for c in (2 * half, 2 * half + 1):
                    nc.tensor.transpose(
                        psv[:, 128 * c : 128 * (c + 1)],
                        sb[f"vT{b}"][:, 128 * c : 128 * (c + 1)],
                        ident,
                    )
                if half == 1:
                    nc.vector.tensor_copy(sb[f"v{b}"], psv)

            return run

        if chunk_major:
            # q/k rows interleaved per eo-chunk: 4 matmuls of ready work per
            # chunk arrival, so the PE never idles (and HAM never
            # re-throttles) while the startup chunks stream in.
            for p in range(3):
                items.append(mm_item(0, p, "qT"))
                items.append(mm_item(1, p, "kT"))
            for p in range(3):
                items.append(mm_item(2, p, "vT"))
            items.append(tr_item(0))
            items.append(tr_item(1))
            return items
        for wi, nm in ((0, "qT"), (1, "kT"), (2, "vT")):
            if wi not in parts:
                continue
            for p in range(3):
                items.append(mm_item(wi, p, nm))
        if 2 in parts:
            items.append(tr_item(0))
            items.append(tr_item(1))
        return items

    # ---- attention for q-block b, interleaving `items` into the pairs ----
    def attn(b, items, stage=None):
        nkt = 4 * (b + 1)          # causal: k tiles 0 .. 4b+3
        npr = nkt // 2
        pso = ps_o.tile([128, 512], f32, tag="o", name=f"pso_{b}")
        psd = ps_d.tile([1, 512], f32, tag="d", name=f"psd_{b}")
        qT = sb[f"qT{b}"]
        sq = 1.0 / (XS * WS) if b in F8 else 1.0

        def off_of(i):
            return 128 * (i - 4 * b) if i >= 4 * b else 0

        def kt_of(i):
            return sb[f"kT{i // 4}"][:, 128 * (i % 4) : 128 * (i % 4 + 1)]

        def v_of(i):
            return sb[f"v{i // 4}"][:, 128 * (i % 4) : 128 * (i % 4 + 1)]

        def emit_scores(g):
            kb = (2 * g) // 4      # pairs never straddle a k-block boundary
            sk = 1.0 / (XS * WS) if kb in F8 else 1.0
            pssc = ps_sc.tile([128, 2, 512], f32, tag="sc", name=f"sc_{b}_{g}")
            for t in range(2):
                i = 2 * g + t
                diag = i >= 4 * b
                off = off_of(i)
                nc.tensor.matmul(
                    pssc[:, t, off:],
                    lhsT=kt_of(i),
                    rhs=qT[:, off:],
                    start=True,
                    stop=not diag,
                )
                if diag:  # add -400*(scale) strictly-upper triangle pre-exp
                    nc.tensor.matmul(
                        pssc[:, t, off : off + 128],
                        lhsT=(triA8 if b in F8 else triA),
                        rhs=ident,
                        start=False,
                        stop=True,
                    )
            moff = off_of(2 * g)
            prb = prb_p.tile([128, 2, 512], bf16, tag="pr", name=f"prb_{b}_{g}")
            nc.scalar.activation(
                prb[:, :, moff:], pssc[:, :, moff:], Exp, scale=SCALE * sq * sk
            )
            return (g, prb)

        def emit_outden(pend, last):
            g, pprb = pend
            for t in range(2):
                i = 2 * g + t
                off = off_of(i)
                nc.tensor.matmul(
                    pso[:, off:],
                    lhsT=v_of(i),
                    rhs=pprb[:, t, off:],
                    start=(i == 0),
                    stop=last and t == 1,
                )
            for t in range(2):
                i = 2 * g + t
                off = off_of(i)
                nc.tensor.matmul(
                    psd[:, off:],
                    lhsT=ones1,
                    rhs=pprb[:, t, off:],
                    start=(i == 0),
                    stop=last and t == 1,
                )
            # cols [0:256] final once the off=128 diag tile has run: drain
            # early so the tail copy+DMA overlaps the last pair. Only the
            # last block DMAs its halves separately (tail latency); the
            # others go out as one 1KB-line transfer on the idle gpsimd
            # queue after the tail copy.
            if g == npr - 2:
                nc.vector.tensor_copy(sb[f"outF{b}"][:, 0:256], pso[:, 0:256])
                if b == NJB - 1:
                    nc.scalar.dma_start(
                        out=outT_ap[:, 512 * b : 512 * b + 256],
                        in_=sb[f"outF{b}"][:, 0:256],
                    )

        # spread items over the early pairs (all before pair npr-2 when
        # possible, so block-3's own k/v items land before they're read).
        spread = max(1, min(npr - 1, 6))
        pipe = []
        for g in range(npr):
            pipe.append(emit_scores(g))
            if g == npr - 1 and stage is not None:
                stage()  # engine-order staged bulk prefetch (scalar queue)
            if g < spread and items:
                budget = (len(items) + (spread - g) - 1) // (spread - g)
                for _ in range(budget):
                    if items:
                        items.pop(0)()
            if len(pipe) > 1:
                emit_outden(pipe.pop(0), last=False)
        while items:  # leftovers (small blocks)
            items.pop(0)()
        while pipe:
            p = pipe.pop(0)
            emit_outden(p, last=not pipe)

        # tail drain: cols [256:512] + this block's denominators
        nc.vector.tensor_copy(sb[f"outF{b}"][:, 256:512], pso[:, 256:])
        if b == NJB - 1:
            nc.sync.dma_start(
                out=outT_ap[:, 512 * b + 256 : 512 * (b + 1)],
                in_=sb[f"outF{b}"][:, 256:512],
            )
        else:
            nc.gpsimd.dma_start(
                out=outT_ap[:, 512 * b : 512 * (b + 1)], in_=sb[f"outF{b}"]
            )
        nc.vector.tensor_copy(sb["denF"][0:1, 512 * b : 512 * (b + 1)], psd)

    # ---- schedule --------------------------------------------------------
    # proj(0): the q row is chunk-paced by the arriving xt0 DMA, so a warm
    # filler matmul sits between consecutive chunk-dependent matmuls to
    # keep the HAM busy-fraction high through the waits; k/v then run as
    # one dense burst on fully resident data.
    ps_q0 = ps_proj.tile([128, 512], f32, tag="proj", name="ps_qT0")
    for eo in range(EO):
        nc.tensor.matmul(
            ps_q0,
            lhsT=sb["w3"][:, eo, 0, :],
            rhs=sb["xt0"][:, eo, :],
            start=(eo == 0),
            stop=(eo == EO - 1),
        )
        if 0 < eo < EO - 1:
            warm_mm()
    nc.vector.tensor_copy(sb["qT0"], ps_q0)
    for it in proj_items(0, parts=(1, 2)):
        it()
    attn(0, proj_items(1),
         stage=lambda: nc.scalar.dma_start(out=sb["xt2"], in_=xt_ap[:, 2]))
    attn(1, proj_items(2),
         stage=lambda: nc.scalar.dma_start(out=sb["xt3"], in_=xt_ap[:, 3]))
    attn(2, proj_items(3, parts=(0, 1)))       # qT/kT of block 3
    attn(3, proj_items(3, parts=(2,)))         # vT + transposes of block 3
    nc.scalar.dma_start(out=den_ap, in_=sb["denF"])


def _build(repeat=1):
    key = ("nc", repeat)
    if key in _CACHE:
        return _CACHE[key]

    import concourse.tile as tile
    from concourse import bacc, mybir

    f32 = mybir.dt.float32
    bf16 = mybir.dt.bfloat16
    fp8 = mybir.dt.float8e4
    nc = bacc.Bacc("TRN2", target_bir_lowering=False, debug=False)

    xt_d = nc.dram_tensor("xt", [128, NJB, EO, 512], bf16, kind="ExternalInput")
    xt8_d = nc.dram_tensor(
        "xt8", [128, len(F8), 3, 2, 512], fp8, kind="ExternalInput"
    )
    w3_d = nc.dram_tensor("w3", [128, EO, 3, H], bf16, kind="ExternalInput")
    w8_d = nc.dram_tensor("w8", [128, 3, 2, 2, H], fp8, kind="ExternalInput")
    consts_d = nc.dram_tensor("consts", [128, 4, 128], bf16, kind="ExternalInput")
    outT_d = nc.dram_tensor("outT", [128, S], bf16, kind="ExternalOutput")
    den_d = nc.dram_tensor("den", [1, S], f32, kind="ExternalOutput")
    dram = (xt_d, xt8_d, w3_d, w8_d, consts_d, outT_d, den_d)

    _CACHE["sb"] = {}
    with tile.TileContext(nc) as tc:
        with (
            tc.tile_pool(name="singles", bufs=1) as singles,
            tc.tile_pool(name="probs", bufs=6) as prb_p,
            tc.tile_pool(name="ps_proj", bufs=2, space="PSUM") as ps_proj,
            tc.tile_pool(name="ps_sc", bufs=2, space="PSUM") as ps_sc,
            tc.tile_pool(name="ps_o", bufs=1, space="PSUM") as ps_o,
            tc.tile_pool(name="ps_d", bufs=1, space="PSUM") as ps_d,
        ):
            pools = (singles, prb_p, ps_proj, ps_sc, ps_o, ps_d)
            for _ in range(repeat):
                _emit_body(nc, tc, pools, dram)

    nc.compile()
    _CACHE[key] = nc
    return nc


def _prep_in_maps(X, padding_mask, W_q, W_k, W_v):
    e4 = ml_dtypes.float8_e4m3
    X = np.asarray(X, dtype=np.float32)
    padding_mask = np.asarray(padding_mask, dtype=np.float32)

    def wprep(W):
        # [H, E] -> [E, H] -> [128(ei), EO, H] with ei innermost of E
        return np.asarray(W, dtype=np.float32).T.reshape(EO, 128, H).transpose(1, 0, 2)

    # [128, EO, 3, H]
    w3 = np.ascontiguousarray(
        np.stack([wprep(W_q), wprep(W_k), wprep(W_v)], axis=2)
    ).astype(ml_dtypes.bfloat16)

    def w8prep(W):
        # [H, E] -> [E, H] -> (3, 2, 128, H) -> [128(ki), 3(c), 2(ko), H]
        a = (np.asarray(W, dtype=np.float32).T * WS).reshape(3, 2, 128, H)
        return a.transpose(2, 0, 1, 3)

    # [128, 3, 2, 2(wi=q,k), H]
    w8 = np.ascontiguousarray(
        np.stack([w8prep(W_q), w8prep(W_k)], axis=3)
    ).astype(e4)

    ident = np.eye(128, dtype=np.float32)
    tri = np.triu(np.ones((128, 128), dtype=np.float32), 1)
    ones = np.ones((128, 128), dtype=np.float32)
    consts = np.ascontiguousarray(
        np.stack(
            [ident, -400.0 * tri, -400.0 * (XS * WS) ** 2 * tri, ones], axis=1
        )
    ).astype(ml_dtypes.bfloat16)  # [128, 4, 128]

    in_maps = []
    for b in range(B):
        Xm = X[b] * padding_mask[b][:, None]  # exact fp32 mask, then quantize
        # [S, E] -> [E, S] -> (3, 2, 128, NJB, 512) -> [128, NJB, 3, 2, 512]
        x8full = (
            (Xm.T * XS)
            .reshape(3, 2, 128, NJB, 512)
            .transpose(2, 3, 0, 1, 4)
        )
        in_maps.append(
            {
                "xt": np.ascontiguousarray(
                    # [S, E] -> [E, S] -> [128(ei), NJB, EO, 512]
                    Xm.T.reshape(EO, 128, NJB, 512).transpose(1, 2, 0, 3)
                ).astype(ml_dtypes.bfloat16),
                "xt8": np.ascontiguousarray(x8full[:, list(F8)]).astype(e4),
                "w3": w3,
                "w8": w8,
                "consts": consts,
            }
        )
    return in_maps


def _finish(res):
    # device wrote outT [128(h), S] bf16 and den [1, S]; out[q, h] = outT.T / den
    return (res["outT"].astype(np.float32).T / res["den"][0][:, None]).astype(
        np.float32
    )


def kernel(X, padding_mask, W_q, W_k, W_v):
    from concourse import bass2jax

    nc = _build(repeat=1)
    in_maps = _prep_in_maps(X, padding_mask, W_q, W_k, W_v)
    results = bass2jax.run_bass_via_pjrt(nc, in_maps, n_cores=B)
    return np.stack([_finish(results[b]) for b in range(B)], axis=0)


# revision 23
# speedup vs baseline: 1.0590x; 1.0590x over previous
"""Causal single-head attention on 8 Trainium2 NeuronCores.

Problem: B=8, S=2048, E=768, HEAD=128, fp32.
  Xm = X * padding_mask[:, :, None]
  q/k/v = Xm @ W_{q,k,v}.T          [B, S, H]
  scores = (q @ k.T) / sqrt(H)  (causal)
  out = softmax(scores) @ v          [B, S, H]

Sharding: pure data-parallel over batch - core b computes batch b; the
tiny projection weights are replicated to every core.

v4 design notes (on top of the bf16 v2 baseline):
  - Startup: inputs stream in ordered globally by first use, with the
    critical sequence (w3[eo01], xt0 in 3 eo-pair chunks, then xt1
    chunks + the fp8 copies) on the sync queue, which serves ~1.5us
    after kernel start at ~250GB/s. Per-chunk completion semaphores let
    each projection matmul wait only on its own chunk. xt2/xt3 bulk is
    issued mid-attention from the scalar engine's program (engine-order
    staging) so it cannot steal startup bandwidth.
  - Projection matmuls for block b+1 are interleaved as small work
    items into the attention pair-pipeline of block b, filling the PE
    bubbles that previously appeared while waiting on ACT exp. Block
    3's projection is split: qT/kT items run inside attn(2), vT +
    transpose items inside attn(3)'s early pairs (legal: pair g only
    touches k-tiles 2g,2g+1, so block-3 k/v tiles are needed only from
    pair 6 on).
  - q/k projections for blocks 2-3 run as fp8(e4m3) DoubleRow matmuls
    (2 contraction elements per cycle, halving their PE time). X and W
    are pre-scaled (x4 / x64) on the host so W escapes the e4m3
    subnormal range; the x65536 score scale folds into the exp scale
    and a second (scaled) causal-mask constant. Host-simulated rel err
    is unchanged (4.8e-3) because the max-error rows live in blocks
    0-1, which stay bf16.
  - Per-block qT/kT/vT/v SBUF tiles so interleaved projection writes
    can never alias attention reads of earlier blocks.
  - outT is drained in bf16 (host divides in fp32); blocks 0-2 go out
    as single 1KB-line transfers on the otherwise-idle gpsimd queue,
    block 3 in two halves on scalar/sync as soon as each half's
    accumulation completes. den goes out as one 8KB DMA at the end.
"""

import math
import sys

import numpy as np

sys.path.insert(0, "/opt/trn_rl_repo")

import ml_dtypes

B, S, E, H = 8, 2048, 768, 128
EO = E // 128          # 6 e-chunks
NJB = S // 512         # 4 q-blocks of 512
SCALE = float(1.0 / math.sqrt(H))
XS, WS = 4.0, 64.0     # fp8 pre-scales for X and W_q/W_k
F8 = (2, 3)            # q-blocks whose q/k projections run in fp8

_CACHE = {}


def _emit_body(nc, tc, pools, dram):
    import concourse.bass as bass  # noqa: F401
    from concourse import mybir

    f32 = mybir.dt.float32
    bf16 = mybir.dt.bfloat16
    fp8 = mybir.dt.float8e4
    DR = mybir.MatmulPerfMode.DoubleRow
    Exp = mybir.ActivationFunctionType.Exp

    singles, prb_p, ps_proj, ps_sc, ps_o, ps_d = pools
    (xt_d, xt8_d, w3_d, w8_d, consts_d, outT_d, den_d) = dram

    sb = _CACHE["sb"]
    if not sb:
        for jb in range(NJB):
            sb[f"xt{jb}"] = singles.tile(
                [128, EO, 512], bf16, tag=f"xt{jb}", name=f"xt{jb}"
            )
            for nm in ("qT", "kT", "vT", "v"):
                sb[f"{nm}{jb}"] = singles.tile(
                    [128, 512], bf16, tag=f"{nm}{jb}", name=f"{nm}{jb}"
                )
            sb[f"outF{jb}"] = singles.tile(
                [128, 512], bf16, tag=f"outF{jb}", name=f"outF{jb}"
            )
        for jb in F8:
            sb[f"xt8_{jb}"] = singles.tile(
                [128, 3, 2, 512], fp8, tag=f"xt8_{jb}", name=f"xt8_{jb}"
            )
        sb["w8"] = singles.tile([128, 3, 2, 2, H], fp8, tag="w8", name="w8")
        sb["w3"] = singles.tile([128, EO, 3, H], bf16, tag="w3", name="w3")
        sb["consts"] = singles.tile([128, 4, 128], bf16, tag="consts", name="consts")
        sb["denF"] = singles.tile([1, S], f32, tag="denF", name="denF")
        sb["warm"] = singles.tile([128, 512], bf16, tag="warm", name="warm")

    xt_ap = xt_d.ap()
    xt8_ap = xt8_d.ap()
    w3_ap = w3_d.ap()
    outT_ap = outT_d.ap()
    den_ap = den_d.ap()

    # ---- prologue loads --------------------------------------------------
    # sync queue: the startup-critical sequence, ordered by first use.
    # scalar joins ~2us later with the rest of w3 + consts; its bulk
    # (xt2/xt3) is staged from inside attn(0)/attn(1) instead. gpsimd
    # stays free for output drains.
    nc.gpsimd.memset(sb["warm"], 0.125)
    nc.sync.dma_start(out=sb["w3"][:, 0:2], in_=w3_ap[:, 0:2])
    for p in range(3):
        nc.sync.dma_start(
            out=sb["xt0"][:, 2 * p : 2 * p + 2], in_=xt_ap[:, 0, 2 * p : 2 * p + 2]
        )
    for p in range(3):
        nc.sync.dma_start(
            out=sb["xt1"][:, 2 * p : 2 * p + 2], in_=xt_ap[:, 1, 2 * p : 2 * p + 2]
        )
    nc.sync.dma_start(out=sb["w8"], in_=w8_d.ap())
    for i, jb in enumerate(F8):
        nc.sync.dma_start(out=sb[f"xt8_{jb}"], in_=xt8_ap[:, i])
    nc.scalar.dma_start(out=sb["w3"][:, 2:4], in_=w3_ap[:, 2:4])
    nc.scalar.dma_start(out=sb["w3"][:, 4:6], in_=w3_ap[:, 4:6])
    nc.scalar.dma_start(out=sb["consts"], in_=consts_d.ap())

    ident = sb["consts"][:, 0, :]
    triA = sb["consts"][:, 1, :]       # -400 (bf16-scale blocks)
    triA8 = sb["consts"][:, 2, :]      # -400 * (XS*WS)^2 (fp8-scale blocks)
    ones1 = sb["consts"][:, 3, 0:1]

    # Short PE warmup bridging the gap until the first chunks land. It
    # deliberately does NOT try to flip the HAM clock-gate early: during
    # the chunk-paced q projection the PE is cold (longer matmuls = a
    # higher busy-fraction across DMA waits), and the dense 12-matmul
    # k/v burst right after the last chunk flips HAM exactly once, with
    # no re-throttle. Early-warm variants lose ~4us to HAM oscillation
    # whenever the startup DMA runs slow.
    ps_warm = ps_proj.tile([128, 512], f32, tag="proj", name="ps_warm")
    for _ in range(5):
        nc.tensor.matmul(
            ps_warm, lhsT=sb["warm"][:, 0:128], rhs=sb["warm"], start=True, stop=True
        )

    # ---- projection work items ------------------------------------------
    # proj(b) = 11 small PE items: 3 per weight (chunk-paced; the last
    # one adds the PSUM->SBUF copy) + 2 transpose items for v. For F8
    # blocks the q/k items are single fp8 DoubleRow matmuls over an
    # eo-pair (256-wide contraction at 2 elems/cycle).
    def proj_items(b, parts=(0, 1, 2), chunk_major=False):
        items = []
        cell = {}
        use8 = b in F8
        x8 = sb[f"xt8_{b}"] if use8 else None

        def mm_item(wi, p, nm, b=b):
            def run():
                if p == 0:
                    cell[wi] = ps_proj.tile(
                        [128, 512], f32, tag="proj", name=f"ps_{nm}{b}"
                    )
                ps = cell[wi]
                if use8 and wi < 2:
                    nc.tensor.matmul(
                        ps,
                        lhsT=sb["w8"][:, p, :, wi, :],
                        rhs=x8[:, p, :, :],
                        start=(p == 0),
                        stop=(p == 2),
                        perf_mode=DR,
                    )
                else:
                    for eo in (2 * p, 2 * p + 1):
                        nc.tensor.matmul(
                            ps,
                            lhsT=sb["w3"][:, eo, wi, :],
                            rhs=sb[f"xt{b}"][:, eo, :],
                            start=(eo == 0),
                            stop=(eo == EO - 1),
                        )
                if p == 2:
                    nc.vector.tensor_copy(sb[f"{nm}{b}"], ps)

            return run

        def tr_item(half, b=b):
            def run():
                if half == 0:
                    cell["psv"] = ps_proj.tile(
                        [128, 512], bf16, tag="proj", name=f"psv{b}"
                    )
                psv = cell["psv"]
                for c in (2 * half, 2 * half + 1):
                    nc.tensor.transpose(
                        psv[:, 128 * c : 128 * (c + 1)],
                        sb[f"vT{b}"][:, 128 * c : 128 * (c + 1)],
                        ident,
                    )
                if half == 1:
                    nc.vector.tensor_copy(sb[f"v{b}"], psv)

            return run

        if chunk_major:
            # q/k rows interleaved per eo-chunk: 4 matmuls of ready work per
            # chunk arrival, so the PE never idles (and HAM never
            # re-throttles) while the startup chunks stream in.
            for p in range(3):
                items.append(mm_item(0, p, "qT"))
                items.append(mm_item(1, p, "kT"))
            for p in range(3):
                items.append(mm_item(2, p, "vT"))
            items.append(tr_item(0))
            items.append(tr_item(1))
            return items
        for wi, nm in ((0, "qT"), (1, "kT"), (2, "vT")):
            if wi not in parts:
                continue
            for p in range(3):
                items.append(mm_item(wi, p, nm))
        if 2 in parts:
            items.append(tr_item(0))
            items.append(tr_item(1))
        return items

    # ---- attention for q-block b, interleaving `items` into the pairs ----
    def attn(b, items, stage=None):
        nkt = 4 * (b + 1)          # causal: k tiles 0 .. 4b+3
        npr = nkt // 2
        pso = ps_o.tile([128, 512], f32, tag="o", name=f"pso_{b}")
        psd = ps_d.tile([1, 512], f32, tag="d", name=f"psd_{b}")
        qT = sb[f"qT{b}"]
        sq = 1.0 / (XS * WS) if b in F8 else 1.0

        def off_of(i):
            return 128 * (i - 4 * b) if i >= 4 * b else 0

        def kt_of(i):
            return sb[f"kT{i // 4}"][:, 128 * (i % 4) : 128 * (i % 4 + 1)]

        def v_of(i):
            return sb[f"v{i // 4}"][:, 128 * (i % 4) : 128 * (i % 4 + 1)]

        def emit_scores(g):
            kb = (2 * g) // 4      # pairs never straddle a k-block boundary
            sk = 1.0 / (XS * WS) if kb in F8 else 1.0
            pssc = ps_sc.tile([128, 2, 512], f32, tag="sc", name=f"sc_{b}_{g}")
            for t in range(2):
                i = 2 * g + t
                diag = i >= 4 * b
                off = off_of(i)
                nc.tensor.matmul(
                    pssc[:, t, off:],
                    lhsT=kt_of(i),
                    rhs=qT[:, off:],
                    start=True,
                    stop=not diag,
                )
                if diag:  # add -400*(scale) strictly-upper triangle pre-exp
                    nc.tensor.matmul(
                        pssc[:, t, off : off + 128],
                        lhsT=(triA8 if b in F8 else triA),
                        rhs=ident,
                        start=False,
                        stop=True,
                    )
            moff = off_of(2 * g)
            prb = prb_p.tile([128, 2, 512], bf16, tag="pr", name=f"prb_{b}_{g}")
            nc.scalar.activation(
                prb[:, :, moff:], pssc[:, :, moff:], Exp, scale=SCALE * sq * sk
            )
            return (g, prb)

        def emit_outden(pend, last):
            g, pprb = pend
            for t in range(2):
                i = 2 * g + t
                off = off_of(i)
                nc.tensor.matmul(
                    pso[:, off:],
                    lhsT=v_of(i),
                    rhs=pprb[:, t, off:],
                    start=(i == 0),
                    stop=last and t == 1,
                )
            for t in range(2):
                i = 2 * g + t
                off = off_of(i)
                nc.tensor.matmul(
                    psd[:, off:],
                    lhsT=ones1,
                    rhs=pprb[:, t, off:],
                    start=(i == 0),
                    stop=last and t == 1,
                )
            # cols [0:256] final once the off=128 diag tile has run: drain
            # early so the tail copy+DMA overlaps the last pair. Only the
            # last block DMAs its halves separately (tail latency); the
            # others go out as one 1KB-line transfer on the idle gpsimd
            # queue after the tail copy.
            if g == npr - 2:
                nc.vector.tensor_copy(sb[f"outF{b}"][:, 0:256], pso[:, 0:256])
                if b == NJB - 1:
                    nc.scalar.dma_start(
                        out=outT_ap[:, 512 * b : 512 * b + 256],
                        in_=sb[f"outF{b}"][:, 0:256],
                    )

        # spread items over the early pairs (all before pair npr-2 when
        # possible, so block-3's own k/v items land before they're read).
        spread = max(1, min(npr - 1, 6))
        pipe = []
        for g in range(npr):
            pipe.append(emit_scores(g))
            if g == npr - 1 and stage is not None:
                stage()  # engine-order staged bulk prefetch (scalar queue)
            if g < spread and items:
                budget = (len(items) + (spread - g) - 1) // (spread - g)
                for _ in range(budget):
                    if items:
                        items.pop(0)()
            if len(pipe) > 1:
                emit_outden(pipe.pop(0), last=False)
        while items:  # leftovers (small blocks)
            items.pop(0)()
        while pipe:
            p = pipe.pop(0)
            emit_outden(p, last=not pipe)

        # tail drain: cols [256:512] + this block's denominators
        nc.vector.tensor_copy(sb[f"outF{b}"][:, 256:512], pso[:, 256:])
        if b == NJB - 1:
            nc.sync.dma_start(
                out=outT_ap[:, 512 * b + 256 : 512 * (b + 1)],
                in_=sb[f"outF{b}"][:, 256:512],
            )
        else:
            nc.gpsimd.dma_start(
                out=outT_ap[:, 512 * b : 512 * (b + 1)], in_=sb[f"outF{b}"]
            )
        nc.vector.tensor_copy(sb["denF"][0:1, 512 * b : 512 * (b + 1)], psd)

    # ---- schedule --------------------------------------------------------
    # proj(0) wi-major: q chunk-paced by the arriving DMA, then k/v as
    # one long dense burst (the HAM warm-up trigger, see above).
    for it in proj_items(0):
        it()
    attn(0, proj_items(1),
         stage=lambda: nc.scalar.dma_start(out=sb["xt2"], in_=xt_ap[:, 2]))
    attn(1, proj_items(2),
         stage=lambda: nc.scalar.dma_start(out=sb["xt3"], in_=xt_ap[:, 3]))
    attn(2, proj_items(3, parts=(0, 1)))       # qT/kT of block 3
    attn(3, proj_items(3, parts=(2,)))         # vT + transposes of block 3
    nc.scalar.dma_start(out=den_ap, in_=sb["denF"])


def _build(repeat=1):
    key = ("nc", repeat)
    if key in _CACHE:
        return _CACHE[key]

    import concourse.tile as tile
    from concourse import bacc, mybir

    f32 = mybir.dt.float32
    bf16 = mybir.dt.bfloat16
    fp8 = mybir.dt.float8e4
    nc = bacc.Bacc("TRN2", target_bir_lowering=False, debug=False)

    xt_d = nc.dram_tensor("xt", [128, NJB, EO, 512], bf16, kind="ExternalInput")
    xt8_d = nc.dram_tensor(
        "xt8", [128, len(F8), 3, 2, 512], fp8, kind="ExternalInput"
    )
    w3_d = nc.dram_tensor("w3", [128, EO, 3, H], bf16, kind="ExternalInput")
    w8_d = nc.dram_tensor("w8", [128, 3, 2, 2, H], fp8, kind="ExternalInput")
    consts_d = nc.dram_tensor("consts", [128, 4, 128], bf16, kind="ExternalInput")
    outT_d = nc.dram_tensor("outT", [128, S], bf16, kind="ExternalOutput")
    den_d = nc.dram_tensor("den", [1, S], f32, kind="ExternalOutput")
    dram = (xt_d, xt8_d, w3_d, w8_d, consts_d, outT_d, den_d)

    _CACHE["sb"] = {}
    with tile.TileContext(nc) as tc:
        with (
            tc.tile_pool(name="singles", bufs=1) as singles,
            tc.tile_pool(name="probs", bufs=6) as prb_p,
            tc.tile_pool(name="ps_proj", bufs=2, space="PSUM") as ps_proj,
            tc.tile_pool(name="ps_sc", bufs=2, space="PSUM") as ps_sc,
            tc.tile_pool(name="ps_o", bufs=1, space="PSUM") as ps_o,
            tc.tile_pool(name="ps_d", bufs=1, space="PSUM") as ps_d,
        ):
            pools = (singles, prb_p, ps_proj, ps_sc, ps_o, ps_d)
            for _ in range(repeat):
                _emit_body(nc, tc, pools, dram)

    nc.compile()
    _CACHE[key] = nc
    return nc


def _prep_in_maps(X, padding_mask, W_q, W_k, W_v):
    e4 = ml_dtypes.float8_e4m3
    X = np.asarray(X, dtype=np.float32)
    padding_mask = np.asarray(padding_mask, dtype=np.float32)

    def wprep(W):
        # [H, E] -> [E, H] -> [128(ei), EO, H] with ei innermost of E
        return np.asarray(W, dtype=np.float32).T.reshape(EO, 128, H).transpose(1, 0, 2)

    # [128, EO, 3, H]
    w3 = np.ascontiguousarray(
        np.stack([wprep(W_q), wprep(W_k), wprep(W_v)], axis=2)
    ).astype(ml_dtypes.bfloat16)

    def w8prep(W):
        # [H, E] -> [E, H] -> (3, 2, 128, H) -> [128(ki), 3(c), 2(ko), H]
        a = (np.asarray(W, dtype=np.float32).T * WS).reshape(3, 2, 128, H)
        return a.transpose(2, 0, 1, 3)

    # [128, 3, 2, 2(wi=q,k), H]
    w8 = np.ascontiguousarray(
        np.stack([w8prep(W_q), w8prep(W_k)], axis=3)
    ).astype(e4)

    ident = np.eye(128, dtype=np.float32)
    tri = np.triu(np.ones((128, 128), dtype=np.float32), 1)
    ones = np.ones((128, 128), dtype=np.float32)
    consts = np.ascontiguousarray(
        np.stack(
            [ident, -400.0 * tri, -400.0 * (XS * WS) ** 2 * tri, ones], axis=1
        )
    ).astype(ml_dtypes.bfloat16)  # [128, 4, 128]

    in_maps = []
    for b in range(B):
        Xm = X[b] * padding_mask[b][:, None]  # exact fp32 mask, then quantize
        # [S, E] -> [E, S] -> (3, 2, 128, NJB, 512) -> [128, NJB, 3, 2, 512]
        x8full = (
            (Xm.T * XS)
            .reshape(3, 2, 128, NJB, 512)
            .transpose(2, 3, 0, 1, 4)
        )
        in_maps.append(
            {
                "xt": np.ascontiguousarray(
                    # [S, E] -> [E, S] -> [128(ei), NJB, EO, 512]
                    Xm.T.reshape(EO, 128, NJB, 512).transpose(1, 2, 0, 3)
                ).astype(ml_dtypes.bfloat16),
                "xt8": np.ascontiguousarray(x8full[:, list(F8)]).astype(e4),
                "w3": w3,
                "w8": w8,
                "consts": consts,
            }
        )
    return in_maps


def _finish(res):
    # device wrote outT [128(h), S] bf16 and den [1, S]; out[q, h] = outT.T / den
    return (res["outT"].astype(np.float32).T / res["den"][0][:, None]).astype(
        np.float32
    )


def kernel(X, padding_mask, W_q, W_k, W_v):
    from concourse import bass2jax

    nc = _build(repeat=1)
    in_maps = _prep_in_maps(X, padding_mask, W_q, W_k, W_v)
    results = bass2jax.run_bass_via_pjrt(nc, in_maps, n_cores=B)
    return np.stack([_finish(results[b]) for b in range(B)], axis=0)


# revision 32
# speedup vs baseline: 1.1217x; 1.0592x over previous
"""Causal single-head attention on 8 Trainium2 NeuronCores.

Problem: B=8, S=2048, E=768, HEAD=128, fp32.
  Xm = X * padding_mask[:, :, None]
  q/k/v = Xm @ W_{q,k,v}.T          [B, S, H]
  scores = (q @ k.T) / sqrt(H)  (causal)
  out = softmax(scores) @ v          [B, S, H]

Sharding: pure data-parallel over batch - core b computes batch b; the
tiny projection weights are replicated to every core.

v4 design notes (on top of the bf16 v2 baseline):
  - Startup: inputs stream in ordered globally by first use, with the
    critical sequence (w3[eo01], xt0 in 3 eo-pair chunks, then xt1
    chunks + the fp8 copies) on the sync queue, which serves ~1.5us
    after kernel start at ~250GB/s. Per-chunk completion semaphores let
    each projection matmul wait only on its own chunk. xt2/xt3 bulk is
    issued mid-attention from the scalar engine's program (engine-order
    staging) so it cannot steal startup bandwidth.
  - Projection matmuls for block b+1 are interleaved as small work
    items into the attention pair-pipeline of block b, filling the PE
    bubbles that previously appeared while waiting on ACT exp. Block
    3's projection is split: qT/kT items run inside attn(2), vT +
    transpose items inside attn(3)'s early pairs (legal: pair g only
    touches k-tiles 2g,2g+1, so block-3 k/v tiles are needed only from
    pair 6 on).
  - q/k projections for blocks 2-3 run as fp8(e4m3) DoubleRow matmuls
    (2 contraction elements per cycle, halving their PE time). X and W
    are pre-scaled (x4 / x64) on the host so W escapes the e4m3
    subnormal range; the x65536 score scale folds into the exp scale
    and a second (scaled) causal-mask constant. Host-simulated rel err
    is unchanged (4.8e-3) because the max-error rows live in blocks
    0-1, which stay bf16.
  - Per-block qT/kT/vT/v SBUF tiles so interleaved projection writes
    can never alias attention reads of earlier blocks.
  - outT is drained in bf16 (host divides in fp32); blocks 0-2 go out
    as single 1KB-line transfers on the otherwise-idle gpsimd queue,
    block 3 in two halves on scalar/sync as soon as each half's
    accumulation completes. den goes out as one 8KB DMA at the end.
"""

import math
import sys

import numpy as np

sys.path.insert(0, "/opt/trn_rl_repo")

import ml_dtypes

B, S, E, H = 8, 2048, 768, 128
EO = E // 128          # 6 e-chunks
NJB = S // 512         # 4 q-blocks of 512
SCALE = float(1.0 / math.sqrt(H))
XS, WS = 4.0, 64.0     # fp8 pre-scales for X and W_q/W_k
F8 = (2, 3)            # q-blocks whose q/k projections run in fp8

_CACHE = {}


def _emit_body(nc, tc, pools, dram):
    import concourse.bass as bass  # noqa: F401
    from concourse import mybir

    f32 = mybir.dt.float32
    bf16 = mybir.dt.bfloat16
    fp8 = mybir.dt.float8e4
    DR = mybir.MatmulPerfMode.DoubleRow
    Exp = mybir.ActivationFunctionType.Exp

    singles, prb_p, prb8_p, ps_proj, ps_sc, ps_o, ps_d = pools
    (xt_d, xt8_d, w3_d, w8_d, consts_d, outT_d, den_d) = dram

    sb = _CACHE["sb"]
    if not sb:
        for jb in range(NJB):
            sb[f"xt{jb}"] = singles.tile(
                [128, EO, 512], bf16, tag=f"xt{jb}", name=f"xt{jb}"
            )
            for nm in ("qT", "kT", "vT"):
                sb[f"{nm}{jb}"] = singles.tile(
                    [128, 512], bf16, tag=f"{nm}{jb}", name=f"{nm}{jb}"
                )
            # v in [128, c, t, h] layout so a k-tile pair (for DoubleRow)
            # is the 3D slice [:, c, :, :]
            if jb not in F8:
                sb[f"v{jb}"] = singles.tile(
                    [128, 2, 2, 128], bf16, tag=f"v{jb}", name=f"v{jb}"
                )
            sb[f"v8_{jb}"] = singles.tile(
                [128, 2, 2, 128], fp8, tag=f"v8_{jb}", name=f"v8_{jb}"
            )
            sb[f"outF{jb}"] = singles.tile(
                [128, 512], bf16, tag=f"outF{jb}", name=f"outF{jb}"
            )
        for jb in F8:
            sb[f"xt8_{jb}"] = singles.tile(
                [128, 3, 2, 512], fp8, tag=f"xt8_{jb}", name=f"xt8_{jb}"
            )
        # [128, 2, 16]: the DoubleRow LDW ISA check requires the Ko step
        # to be a multiple of 16 bytes, so the ones column is padded.
        sb["ones8"] = singles.tile([128, 2, 16], fp8, tag="ones8", name="ones8")
        sb["w8"] = singles.tile([128, 3, 2, 2, H], fp8, tag="w8", name="w8")
        sb["w3"] = singles.tile([128, EO, 3, H], bf16, tag="w3", name="w3")
        sb["consts"] = singles.tile([128, 4, 128], bf16, tag="consts", name="consts")
        sb["denF"] = singles.tile([1, S], f32, tag="denF", name="denF")
        sb["warm"] = singles.tile([128, 512], bf16, tag="warm", name="warm")

    xt_ap = xt_d.ap()
    xt8_ap = xt8_d.ap()
    w3_ap = w3_d.ap()
    outT_ap = outT_d.ap()
    den_ap = den_d.ap()

    # ---- prologue loads --------------------------------------------------
    # sync queue: the startup-critical sequence, ordered by first use.
    # scalar joins ~2us later with the rest of w3 + consts; its bulk
    # (xt2/xt3) is staged from inside attn(0)/attn(1) instead. gpsimd
    # stays free for output drains.
    nc.gpsimd.memset(sb["warm"], 0.125)
    nc.gpsimd.memset(sb["ones8"], 1.0)
    nc.sync.dma_start(out=sb["w3"][:, 0:2], in_=w3_ap[:, 0:2])
    for p in range(3):
        nc.sync.dma_start(
            out=sb["xt0"][:, 2 * p : 2 * p + 2], in_=xt_ap[:, 0, 2 * p : 2 * p + 2]
        )
    for p in range(3):
        nc.sync.dma_start(
            out=sb["xt1"][:, 2 * p : 2 * p + 2], in_=xt_ap[:, 1, 2 * p : 2 * p + 2]
        )
    nc.sync.dma_start(out=sb["w8"], in_=w8_d.ap())
    for i, jb in enumerate(F8):
        nc.sync.dma_start(out=sb[f"xt8_{jb}"], in_=xt8_ap[:, i])
    nc.scalar.dma_start(out=sb["w3"][:, 2:4], in_=w3_ap[:, 2:4])
    nc.scalar.dma_start(out=sb["w3"][:, 4:6], in_=w3_ap[:, 4:6])
    nc.scalar.dma_start(out=sb["consts"], in_=consts_d.ap())

    ident = sb["consts"][:, 0, :]
    triA = sb["consts"][:, 1, :]       # -400 (bf16-scale blocks)
    triA8 = sb["consts"][:, 2, :]      # -400 * (XS*WS)^2 (fp8-scale blocks)
    ones1 = sb["consts"][:, 3, 0:1]

    # Short PE warmup bridging the gap until the first chunks land. It
    # deliberately does NOT try to flip the HAM clock-gate early: during
    # the chunk-paced q projection the PE is cold (longer matmuls = a
    # higher busy-fraction across DMA waits), and the dense 12-matmul
    # k/v burst right after the last chunk flips HAM exactly once, with
    # no re-throttle. Early-warm variants lose ~4us to HAM oscillation
    # whenever the startup DMA runs slow.
    ps_warm = ps_proj.tile([128, 512], f32, tag="proj", name="ps_warm")
    for _ in range(5):
        nc.tensor.matmul(
            ps_warm, lhsT=sb["warm"][:, 0:128], rhs=sb["warm"], start=True, stop=True
        )

    # ---- projection work items ------------------------------------------
    # proj(b) = 11 small PE items: 3 per weight (chunk-paced; the last
    # one adds the PSUM->SBUF copy) + 2 transpose items for v. For F8
    # blocks the q/k items are single fp8 DoubleRow matmuls over an
    # eo-pair (256-wide contraction at 2 elems/cycle).
    def proj_items(b, parts=(0, 1, 2), chunk_major=False):
        items = []
        cell = {}
        use8 = b in F8
        x8 = sb[f"xt8_{b}"] if use8 else None

        def mm_item(wi, p, nm, b=b):
            def run():
                if p == 0:
                    cell[wi] = ps_proj.tile(
                        [128, 512], f32, tag="proj", name=f"ps_{nm}{b}"
                    )
                ps = cell[wi]
                if use8 and wi < 2:
                    nc.tensor.matmul(
                        ps,
                        lhsT=sb["w8"][:, p, :, wi, :],
                        rhs=x8[:, p, :, :],
                        start=(p == 0),
                        stop=(p == 2),
                        perf_mode=DR,
                    )
                else:
                    for eo in (2 * p, 2 * p + 1):
                        nc.tensor.matmul(
                            ps,
                            lhsT=sb["w3"][:, eo, wi, :],
                            rhs=sb[f"xt{b}"][:, eo, :],
                            start=(eo == 0),
                            stop=(eo == EO - 1),
                        )
                if p == 2:
                    nc.vector.tensor_copy(sb[f"{nm}{b}"], ps)

            return run

        def tr_item(half, b=b):
            def run():
                if half == 0:
                    cell["psv"] = ps_proj.tile(
                        [128, 2, 2, 128], bf16, tag="proj", name=f"psv{b}"
                    )
                psv = cell["psv"]
                for c in (2 * half, 2 * half + 1):
                    nc.tensor.transpose(
                        psv[:, c // 2, c % 2, :],
                        sb[f"vT{b}"][:, 128 * c : 128 * (c + 1)],
                        ident,
                    )
                if half == 1:
                    # fp8 copy always (attn(2)/(3) read every block's v in
                    # fp8); bf16 copy only for blocks attn(0)/(1) touch.
                    nc.vector.tensor_copy(sb[f"v8_{b}"], psv)
                    if b not in F8:
                        nc.vector.tensor_copy(sb[f"v{b}"], psv)

            return run

        if chunk_major:
            # q/k rows interleaved per eo-chunk: 4 matmuls of ready work per
            # chunk arrival, so the PE never idles (and HAM never
            # re-throttles) while the startup chunks stream in.
            for p in range(3):
                items.append(mm_item(0, p, "qT"))
                items.append(mm_item(1, p, "kT"))
            for p in range(3):
                items.append(mm_item(2, p, "vT"))
            items.append(tr_item(0))
            items.append(tr_item(1))
            return items
        for wi, nm in ((0, "qT"), (1, "kT"), (2, "vT")):
            if wi not in parts:
                continue
            for p in range(3):
                items.append(mm_item(wi, p, nm))
        if 2 in parts:
            items.append(tr_item(0))
            items.append(tr_item(1))
        return items

    # ---- attention for q-block b, interleaving `items` into the pairs ----
    def attn(b, items, stage=None):
        nkt = 4 * (b + 1)          # causal: k tiles 0 .. 4b+3
        npr = nkt // 2
        pso = ps_o.tile([128, 512], f32, tag="o", name=f"pso_{b}")
        psd = ps_d.tile([1, 512], f32, tag="d", name=f"psd_{b}")
        qT = sb[f"qT{b}"]
        sq = 1.0 / (XS * WS) if b in F8 else 1.0

        def off_of(i):
            return 128 * (i - 4 * b) if i >= 4 * b else 0

        use8 = b in F8            # fp8 probs/v + DoubleRow out/den
        vpfx = "v8_" if use8 else "v"

        def kt_of(i):
            return sb[f"kT{i // 4}"][:, 128 * (i % 4) : 128 * (i % 4 + 1)]

        def v_of(i):
            return sb[f"{vpfx}{i // 4}"][:, (i % 4) // 2, (i % 4) % 2, :]

        def emit_scores(g):
            kb = (2 * g) // 4      # pairs never straddle a k-block boundary
            sk = 1.0 / (XS * WS) if kb in F8 else 1.0
            pssc = ps_sc.tile([128, 2, 512], f32, tag="sc", name=f"sc_{b}_{g}")
            for t in range(2):
                i = 2 * g + t
                diag = i >= 4 * b
                off = off_of(i)
                nc.tensor.matmul(
                    pssc[:, t, off:],
                    lhsT=kt_of(i),
                    rhs=qT[:, off:],
                    start=True,
                    stop=not diag,
                )
                if diag:  # add -400*(scale) strictly-upper triangle pre-exp
                    nc.tensor.matmul(
                        pssc[:, t, off : off + 128],
                        lhsT=(triA8 if b in F8 else triA),
                        rhs=ident,
                        start=False,
                        stop=True,
                    )
            moff = off_of(2 * g)
            if use8:
                prb = prb8_p.tile([128, 2, 512], fp8, tag="pr8", name=f"prb_{b}_{g}")
            else:
                prb = prb_p.tile([128, 2, 512], bf16, tag="pr", name=f"prb_{b}_{g}")
            nc.scalar.activation(
                prb[:, :, moff:], pssc[:, :, moff:], Exp, scale=SCALE * sq * sk
            )
            return (g, prb)

        def emit_outden(pend, last):
            g, pprb = pend
            if use8 and 2 * g + 1 < 4 * b:
                # non-diag pair, fp8: one DoubleRow matmul contracts both
                # k-tiles of the pair at 2 elements/cycle (for out and den)
                kb, c = (2 * g) // 4, ((2 * g) % 4) // 2
                nc.tensor.matmul(
                    pso,
                    lhsT=sb[f"v8_{kb}"][:, c, :, :],
                    rhs=pprb,
                    start=(g == 0),
                    stop=False,
                    perf_mode=DR,
                )
                nc.tensor.matmul(
                    psd,
                    lhsT=sb["ones8"][:, :, 0:1],
                    rhs=pprb,
                    start=(g == 0),
                    stop=False,
                    perf_mode=DR,
                )
            else:
                dlhs = sb["ones8"][:, 0, 0:1] if use8 else ones1
                for t in range(2):
                    i = 2 * g + t
                    off = off_of(i)
                    nc.tensor.matmul(
                        pso[:, off:],
                        lhsT=v_of(i),
                        rhs=pprb[:, t, off:],
                        start=(i == 0),
                        stop=last and t == 1,
                    )
                for t in range(2):
                    i = 2 * g + t
                    off = off_of(i)
                    nc.tensor.matmul(
                        psd[:, off:],
                        lhsT=dlhs,
                        rhs=pprb[:, t, off:],
                        start=(i == 0),
                        stop=last and t == 1,
                    )
            # cols [0:256] final once the off=128 diag tile has run: drain
            # early so the tail copy+DMA overlaps the last pair. Only the
            # last block DMAs its halves separately (tail latency); the
            # others go out as one 1KB-line transfer on the idle gpsimd
            # queue after the tail copy.
            if g == npr - 2:
                nc.vector.tensor_copy(sb[f"outF{b}"][:, 0:256], pso[:, 0:256])
                if b == NJB - 1:
                    nc.scalar.dma_start(
                        out=outT_ap[:, 512 * b : 512 * b + 256],
                        in_=sb[f"outF{b}"][:, 0:256],
                    )

        # spread items over the early pairs (all before pair npr-2 when
        # possible, so block-3's own k/v items land before they're read).
        spread = max(1, min(npr - 1, 6))
        pipe = []
        for g in range(npr):
            pipe.append(emit_scores(g))
            if g == npr - 1 and stage is not None:
                stage()  # engine-order staged bulk prefetch (scalar queue)
            if g < spread and items:
                budget = (len(items) + (spread - g) - 1) // (spread - g)
                for _ in range(budget):
                    if items:
                        items.pop(0)()
            if len(pipe) > 1:
                emit_outden(pipe.pop(0), last=False)
        while items:  # leftovers (small blocks)
            items.pop(0)()
        while pipe:
            p = pipe.pop(0)
            emit_outden(p, last=not pipe)

        # tail drain: cols [256:512] + this block's denominators
        nc.vector.tensor_copy(sb[f"outF{b}"][:, 256:512], pso[:, 256:])
        if b == NJB - 1:
            nc.sync.dma_start(
                out=outT_ap[:, 512 * b + 256 : 512 * (b + 1)],
                in_=sb[f"outF{b}"][:, 256:512],
            )
        else:
            nc.gpsimd.dma_start(
                out=outT_ap[:, 512 * b : 512 * (b + 1)], in_=sb[f"outF{b}"]
            )
        nc.vector.tensor_copy(sb["denF"][0:1, 512 * b : 512 * (b + 1)], psd)

    # ---- schedule --------------------------------------------------------
    # proj(0) wi-major: q chunk-paced by the arriving DMA, then k/v as
    # one long dense burst (the HAM warm-up trigger, see above).
    for it in proj_items(0):
        it()
    attn(0, proj_items(1),
         stage=lambda: nc.scalar.dma_start(out=sb["xt2"], in_=xt_ap[:, 2]))
    attn(1, proj_items(2),
         stage=lambda: nc.scalar.dma_start(out=sb["xt3"], in_=xt_ap[:, 3]))
    attn(2, proj_items(3, parts=(0, 1)))       # qT/kT of block 3
    attn(3, proj_items(3, parts=(2,)))         # vT + transposes of block 3
    nc.scalar.dma_start(out=den_ap, in_=sb["denF"])


def _build(repeat=1):
    key = ("nc", repeat)
    if key in _CACHE:
        return _CACHE[key]

    import concourse.tile as tile
    from concourse import bacc, mybir

    f32 = mybir.dt.float32
    bf16 = mybir.dt.bfloat16
    fp8 = mybir.dt.float8e4
    nc = bacc.Bacc("TRN2", target_bir_lowering=False, debug=False)

    xt_d = nc.dram_tensor("xt", [128, NJB, EO, 512], bf16, kind="ExternalInput")
    xt8_d = nc.dram_tensor(
        "xt8", [128, len(F8), 3, 2, 512], fp8, kind="ExternalInput"
    )
    w3_d = nc.dram_tensor("w3", [128, EO, 3, H], bf16, kind="ExternalInput")
    w8_d = nc.dram_tensor("w8", [128, 3, 2, 2, H], fp8, kind="ExternalInput")
    consts_d = nc.dram_tensor("consts", [128, 4, 128], bf16, kind="ExternalInput")
    outT_d = nc.dram_tensor("outT", [128, S], bf16, kind="ExternalOutput")
    den_d = nc.dram_tensor("den", [1, S], f32, kind="ExternalOutput")
    dram = (xt_d, xt8_d, w3_d, w8_d, consts_d, outT_d, den_d)

    _CACHE["sb"] = {}
    with tile.TileContext(nc) as tc:
        with (
            tc.tile_pool(name="singles", bufs=1) as singles,
            tc.tile_pool(name="probs", bufs=6) as prb_p,
            tc.tile_pool(name="probs8", bufs=6) as prb8_p,
            tc.tile_pool(name="ps_proj", bufs=2, space="PSUM") as ps_proj,
            tc.tile_pool(name="ps_sc", bufs=2, space="PSUM") as ps_sc,
            tc.tile_pool(name="ps_o", bufs=1, space="PSUM") as ps_o,
            tc.tile_pool(name="ps_d", bufs=1, space="PSUM") as ps_d,
        ):
            pools = (singles, prb_p, prb8_p, ps_proj, ps_sc, ps_o, ps_d)
            for _ in range(repeat):
                _emit_body(nc, tc, pools, dram)

    nc.compile()
    _CACHE[key] = nc
    return nc


def _prep_in_maps(X, padding_mask, W_q, W_k, W_v):
    e4 = ml_dtypes.float8_e4m3
    X = np.asarray(X, dtype=np.float32)
    padding_mask = np.asarray(padding_mask, dtype=np.float32)

    def wprep(W):
        # [H, E] -> [E, H] -> [128(ei), EO, H] with ei innermost of E
        return np.asarray(W, dtype=np.float32).T.reshape(EO, 128, H).transpose(1, 0, 2)

    # [128, EO, 3, H]
    w3 = np.ascontiguousarray(
        np.stack([wprep(W_q), wprep(W_k), wprep(W_v)], axis=2)
    ).astype(ml_dtypes.bfloat16)

    def w8prep(W):
        # [H, E] -> [E, H] -> (3, 2, 128, H) -> [128(ki), 3(c), 2(ko), H]
        a = (np.asarray(W, dtype=np.float32).T * WS).reshape(3, 2, 128, H)
        return a.transpose(2, 0, 1, 3)

    # [128, 3, 2, 2(wi=q,k), H]
    w8 = np.ascontiguousarray(
        np.stack([w8prep(W_q), w8prep(W_k)], axis=3)
    ).astype(e4)

    ident = np.eye(128, dtype=np.float32)
    tri = np.triu(np.ones((128, 128), dtype=np.float32), 1)
    ones = np.ones((128, 128), dtype=np.float32)
    consts = np.ascontiguousarray(
        np.stack(
            [ident, -400.0 * tri, -400.0 * (XS * WS) ** 2 * tri, ones], axis=1
        )
    ).astype(ml_dtypes.bfloat16)  # [128, 4, 128]

    in_maps = []
    for b in range(B):
        Xm = X[b] * padding_mask[b][:, None]  # exact fp32 mask, then quantize
        # [S, E] -> [E, S] -> (3, 2, 128, NJB, 512) -> [128, NJB, 3, 2, 512]
        x8full = (
            (Xm.T * XS)
            .reshape(3, 2, 128, NJB, 512)
            .transpose(2, 3, 0, 1, 4)
        )
        in_maps.append(
            {
                "xt": np.ascontiguousarray(
                    # [S, E] -> [E, S] -> [128(ei), NJB, EO, 512]
                    Xm.T.reshape(EO, 128, NJB, 512).transpose(1, 2, 0, 3)
                ).astype(ml_dtypes.bfloat16),
                "xt8": np.ascontiguousarray(x8full[:, list(F8)]).astype(e4),
                "w3": w3,
                "w8": w8,
                "consts": consts,
            }
        )
    return in_maps


def _finish(res):
    # device wrote outT [128(h), S] bf16 and den [1, S]; out[q, h] = outT.T / den
    return (res["outT"].astype(np.float32).T / res["den"][0][:, None]).astype(
        np.float32
    )


def kernel(X, padding_mask, W_q, W_k, W_v):
    from concourse import bass2jax

    nc = _build(repeat=1)
    in_maps = _prep_in_maps(X, padding_mask, W_q, W_k, W_v)
    results = bass2jax.run_bass_via_pjrt(nc, in_maps, n_cores=B)
    return np.stack([_finish(results[b]) for b in range(B)], axis=0)


# revision 33
# speedup vs baseline: 1.1785x; 1.0507x over previous
"""Causal single-head attention on 8 Trainium2 NeuronCores.

Problem: B=8, S=2048, E=768, HEAD=128, fp32.
  Xm = X * padding_mask[:, :, None]
  q/k/v = Xm @ W_{q,k,v}.T          [B, S, H]
  scores = (q @ k.T) / sqrt(H)  (causal)
  out = softmax(scores) @ v          [B, S, H]

Sharding: pure data-parallel over batch - core b computes batch b; the
tiny projection weights are replicated to every core.

v4 design notes (on top of the bf16 v2 baseline):
  - Startup: inputs stream in ordered globally by first use, with the
    critical sequence (w3[eo01], xt0 in 3 eo-pair chunks, then xt1
    chunks + the fp8 copies) on the sync queue, which serves ~1.5us
    after kernel start at ~250GB/s. Per-chunk completion semaphores let
    each projection matmul wait only on its own chunk. xt2/xt3 bulk is
    issued mid-attention from the scalar engine's program (engine-order
    staging) so it cannot steal startup bandwidth.
  - Projection matmuls for block b+1 are interleaved as small work
    items into the attention pair-pipeline of block b, filling the PE
    bubbles that previously appeared while waiting on ACT exp. Block
    3's projection is split: qT/kT items run inside attn(2), vT +
    transpose items inside attn(3)'s early pairs (legal: pair g only
    touches k-tiles 2g,2g+1, so block-3 k/v tiles are needed only from
    pair 6 on).
  - q/k projections for blocks 2-3 run as fp8(e4m3) DoubleRow matmuls
    (2 contraction elements per cycle, halving their PE time). X and W
    are pre-scaled (x4 / x64) on the host so W escapes the e4m3
    subnormal range; the x65536 score scale folds into the exp scale
    and a second (scaled) causal-mask constant. Host-simulated rel err
    is unchanged (4.8e-3) because the max-error rows live in blocks
    0-1, which stay bf16.
  - Per-block qT/kT/vT/v SBUF tiles so interleaved projection writes
    can never alias attention reads of earlier blocks.
  - outT is drained in bf16 (host divides in fp32); blocks 0-2 go out
    as single 1KB-line transfers on the otherwise-idle gpsimd queue,
    block 3 in two halves on scalar/sync as soon as each half's
    accumulation completes. den goes out as one 8KB DMA at the end.
"""

import math
import sys

import numpy as np

sys.path.insert(0, "/opt/trn_rl_repo")

import ml_dtypes

B, S, E, H = 8, 2048, 768, 128
EO = E // 128          # 6 e-chunks
NJB = S // 512         # 4 q-blocks of 512
SCALE = float(1.0 / math.sqrt(H))
XS, WS = 4.0, 64.0     # fp8 pre-scales for X and W_q/W_k
F8 = (1, 2, 3)         # q-blocks running the fp8 path (block 0 stays bf16)

_CACHE = {}


def _emit_body(nc, tc, pools, dram):
    import concourse.bass as bass  # noqa: F401
    from concourse import mybir

    f32 = mybir.dt.float32
    bf16 = mybir.dt.bfloat16
    fp8 = mybir.dt.float8e4
    DR = mybir.MatmulPerfMode.DoubleRow
    Exp = mybir.ActivationFunctionType.Exp

    singles, prb_p, prb8_p, ps_proj, ps_sc, ps_o, ps_d = pools
    (xt_d, xt8_d, w3_d, w8_d, consts_d, outT_d, den_d) = dram

    sb = _CACHE["sb"]
    if not sb:
        for jb in range(NJB):
            sb[f"xt{jb}"] = singles.tile(
                [128, EO, 512], bf16, tag=f"xt{jb}", name=f"xt{jb}"
            )
            for nm in ("qT", "kT", "vT"):
                sb[f"{nm}{jb}"] = singles.tile(
                    [128, 512], bf16, tag=f"{nm}{jb}", name=f"{nm}{jb}"
                )
            # v in [128, c, t, h] layout so a k-tile pair (for DoubleRow)
            # is the 3D slice [:, c, :, :]
            if jb not in F8:
                sb[f"v{jb}"] = singles.tile(
                    [128, 2, 2, 128], bf16, tag=f"v{jb}", name=f"v{jb}"
                )
            sb[f"v8_{jb}"] = singles.tile(
                [128, 2, 2, 128], fp8, tag=f"v8_{jb}", name=f"v8_{jb}"
            )
            sb[f"outF{jb}"] = singles.tile(
                [128, 512], bf16, tag=f"outF{jb}", name=f"outF{jb}"
            )
        for jb in F8:
            sb[f"xt8_{jb}"] = singles.tile(
                [128, 3, 2, 512], fp8, tag=f"xt8_{jb}", name=f"xt8_{jb}"
            )
        # [128, 2, 16]: the DoubleRow LDW ISA check requires the Ko step
        # to be a multiple of 16 bytes, so the ones column is padded.
        sb["ones8"] = singles.tile([128, 2, 16], fp8, tag="ones8", name="ones8")
        sb["w8"] = singles.tile([128, 3, 2, 2, H], fp8, tag="w8", name="w8")
        sb["w3"] = singles.tile([128, EO, 3, H], bf16, tag="w3", name="w3")
        sb["consts"] = singles.tile([128, 4, 128], bf16, tag="consts", name="consts")
        sb["denF"] = singles.tile([1, S], f32, tag="denF", name="denF")
        sb["warm"] = singles.tile([128, 512], bf16, tag="warm", name="warm")

    xt_ap = xt_d.ap()
    xt8_ap = xt8_d.ap()
    w3_ap = w3_d.ap()
    outT_ap = outT_d.ap()
    den_ap = den_d.ap()

    # ---- prologue loads --------------------------------------------------
    # sync queue: the startup-critical sequence, ordered by first use.
    # scalar joins ~2us later with the rest of w3 + consts; its bulk
    # (xt2/xt3) is staged from inside attn(0)/attn(1) instead. gpsimd
    # stays free for output drains.
    nc.gpsimd.memset(sb["warm"], 0.125)
    nc.gpsimd.memset(sb["ones8"], 1.0)
    nc.sync.dma_start(out=sb["w3"][:, 0:2], in_=w3_ap[:, 0:2])
    for p in range(3):
        nc.sync.dma_start(
            out=sb["xt0"][:, 2 * p : 2 * p + 2], in_=xt_ap[:, 0, 2 * p : 2 * p + 2]
        )
    for p in range(3):
        nc.sync.dma_start(
            out=sb["xt1"][:, 2 * p : 2 * p + 2], in_=xt_ap[:, 1, 2 * p : 2 * p + 2]
        )
    nc.sync.dma_start(out=sb["w8"], in_=w8_d.ap())
    for i, jb in enumerate(F8):
        nc.sync.dma_start(out=sb[f"xt8_{jb}"], in_=xt8_ap[:, i])
    nc.scalar.dma_start(out=sb["w3"][:, 2:4], in_=w3_ap[:, 2:4])
    nc.scalar.dma_start(out=sb["w3"][:, 4:6], in_=w3_ap[:, 4:6])
    nc.scalar.dma_start(out=sb["consts"], in_=consts_d.ap())

    ident = sb["consts"][:, 0, :]
    triA = sb["consts"][:, 1, :]       # -400 (bf16-scale blocks)
    triA8 = sb["consts"][:, 2, :]      # -400 * (XS*WS)^2 (fp8-scale blocks)
    ones1 = sb["consts"][:, 3, 0:1]

    # Short PE warmup bridging the gap until the first chunks land. It
    # deliberately does NOT try to flip the HAM clock-gate early: during
    # the chunk-paced q projection the PE is cold (longer matmuls = a
    # higher busy-fraction across DMA waits), and the dense 12-matmul
    # k/v burst right after the last chunk flips HAM exactly once, with
    # no re-throttle. Early-warm variants lose ~4us to HAM oscillation
    # whenever the startup DMA runs slow.
    ps_warm = ps_proj.tile([128, 512], f32, tag="proj", name="ps_warm")
    for _ in range(5):
        nc.tensor.matmul(
            ps_warm, lhsT=sb["warm"][:, 0:128], rhs=sb["warm"], start=True, stop=True
        )

    # ---- projection work items ------------------------------------------
    # proj(b) = 11 small PE items: 3 per weight (chunk-paced; the last
    # one adds the PSUM->SBUF copy) + 2 transpose items for v. For F8
    # blocks the q/k items are single fp8 DoubleRow matmuls over an
    # eo-pair (256-wide contraction at 2 elems/cycle).
    def proj_items(b, parts=(0, 1, 2), chunk_major=False):
        items = []
        cell = {}
        use8 = b in F8
        x8 = sb[f"xt8_{b}"] if use8 else None

        def mm_item(wi, p, nm, b=b):
            def run():
                if p == 0:
                    cell[wi] = ps_proj.tile(
                        [128, 512], f32, tag="proj", name=f"ps_{nm}{b}"
                    )
                ps = cell[wi]
                if use8 and wi < 2:
                    nc.tensor.matmul(
                        ps,
                        lhsT=sb["w8"][:, p, :, wi, :],
                        rhs=x8[:, p, :, :],
                        start=(p == 0),
                        stop=(p == 2),
                        perf_mode=DR,
                    )
                else:
                    for eo in (2 * p, 2 * p + 1):
                        nc.tensor.matmul(
                            ps,
                            lhsT=sb["w3"][:, eo, wi, :],
                            rhs=sb[f"xt{b}"][:, eo, :],
                            start=(eo == 0),
                            stop=(eo == EO - 1),
                        )
                if p == 2:
                    nc.vector.tensor_copy(sb[f"{nm}{b}"], ps)

            return run

        def tr_item(half, b=b):
            def run():
                if half == 0:
                    cell["psv"] = ps_proj.tile(
                        [128, 2, 2, 128], bf16, tag="proj", name=f"psv{b}"
                    )
                psv = cell["psv"]
                for c in (2 * half, 2 * half + 1):
                    nc.tensor.transpose(
                        psv[:, c // 2, c % 2, :],
                        sb[f"vT{b}"][:, 128 * c : 128 * (c + 1)],
                        ident,
                    )
                if half == 1:
                    # fp8 copy always (attn(2)/(3) read every block's v in
                    # fp8); bf16 copy only for blocks attn(0)/(1) touch.
                    nc.vector.tensor_copy(sb[f"v8_{b}"], psv)
                    if b not in F8:
                        nc.vector.tensor_copy(sb[f"v{b}"], psv)

            return run

        if chunk_major:
            # q/k rows interleaved per eo-chunk: 4 matmuls of ready work per
            # chunk arrival, so the PE never idles (and HAM never
            # re-throttles) while the startup chunks stream in.
            for p in range(3):
                items.append(mm_item(0, p, "qT"))
                items.append(mm_item(1, p, "kT"))
            for p in range(3):
                items.append(mm_item(2, p, "vT"))
            items.append(tr_item(0))
            items.append(tr_item(1))
            return items
        worder = ((2, "vT"), (0, "qT"), (1, "kT")) if use8 else (
            (0, "qT"), (1, "kT"), (2, "vT"))
        for wi, nm in worder:
            if wi not in parts:
                continue
            for p in range(3):
                items.append(mm_item(wi, p, nm))
            if wi == 2:
                items.append(tr_item(0))
                items.append(tr_item(1))
        return items

    # ---- attention for q-block b, interleaving `items` into the pairs ----
    def attn(b, items, stage=None):
        nkt = 4 * (b + 1)          # causal: k tiles 0 .. 4b+3
        npr = nkt // 2
        pso = ps_o.tile([128, 512], f32, tag="o", name=f"pso_{b}")
        psd = ps_d.tile([1, 512], f32, tag="d", name=f"psd_{b}")
        qT = sb[f"qT{b}"]
        sq = 1.0 / (XS * WS) if b in F8 else 1.0

        def off_of(i):
            return 128 * (i - 4 * b) if i >= 4 * b else 0

        use8 = b in F8            # fp8 probs/v + DoubleRow out/den
        vpfx = "v8_" if use8 else "v"

        def kt_of(i):
            return sb[f"kT{i // 4}"][:, 128 * (i % 4) : 128 * (i % 4 + 1)]

        def v_of(i):
            return sb[f"{vpfx}{i // 4}"][:, (i % 4) // 2, (i % 4) % 2, :]

        def emit_scores(g):
            kb = (2 * g) // 4      # pairs never straddle a k-block boundary
            sk = 1.0 / (XS * WS) if kb in F8 else 1.0
            pssc = ps_sc.tile([128, 2, 512], f32, tag="sc", name=f"sc_{b}_{g}")
            for t in range(2):
                i = 2 * g + t
                diag = i >= 4 * b
                off = off_of(i)
                nc.tensor.matmul(
                    pssc[:, t, off:],
                    lhsT=kt_of(i),
                    rhs=qT[:, off:],
                    start=True,
                    stop=not diag,
                )
                if diag:  # add -400*(scale) strictly-upper triangle pre-exp
                    nc.tensor.matmul(
                        pssc[:, t, off : off + 128],
                        lhsT=(triA8 if b in F8 else triA),
                        rhs=ident,
                        start=False,
                        stop=True,
                    )
            moff = off_of(2 * g)
            if use8:
                prb = prb8_p.tile([128, 2, 512], fp8, tag="pr8", name=f"prb_{b}_{g}")
            else:
                prb = prb_p.tile([128, 2, 512], bf16, tag="pr", name=f"prb_{b}_{g}")
            nc.scalar.activation(
                prb[:, :, moff:], pssc[:, :, moff:], Exp, scale=SCALE * sq * sk
            )
            return (g, prb)

        def emit_outden(pend, last):
            g, pprb = pend
            if use8 and 2 * g + 1 < 4 * b:
                # non-diag pair, fp8: one DoubleRow matmul contracts both
                # k-tiles of the pair at 2 elements/cycle (for out and den)
                kb, c = (2 * g) // 4, ((2 * g) % 4) // 2
                nc.tensor.matmul(
                    pso,
                    lhsT=sb[f"v8_{kb}"][:, c, :, :],
                    rhs=pprb,
                    start=(g == 0),
                    stop=False,
                    perf_mode=DR,
                )
                nc.tensor.matmul(
                    psd,
                    lhsT=sb["ones8"][:, :, 0:1],
                    rhs=pprb,
                    start=(g == 0),
                    stop=False,
                    perf_mode=DR,
                )
            else:
                dlhs = sb["ones8"][:, 0, 0:1] if use8 else ones1
                for t in range(2):
                    i = 2 * g + t
                    off = off_of(i)
                    nc.tensor.matmul(
                        pso[:, off:],
                        lhsT=v_of(i),
                        rhs=pprb[:, t, off:],
                        start=(i == 0),
                        stop=last and t == 1,
                    )
                for t in range(2):
                    i = 2 * g + t
                    off = off_of(i)
                    nc.tensor.matmul(
                        psd[:, off:],
                        lhsT=dlhs,
                        rhs=pprb[:, t, off:],
                        start=(i == 0),
                        stop=last and t == 1,
                    )
            # cols [0:256] final once the off=128 diag tile has run: drain
            # early so the tail copy+DMA overlaps the last pair. Only the
            # last block DMAs its halves separately (tail latency); the
            # others go out as one 1KB-line transfer on the idle gpsimd
            # queue after the tail copy.
            if g == npr - 2:
                nc.vector.tensor_copy(sb[f"outF{b}"][:, 0:256], pso[:, 0:256])
                if b == NJB - 1:
                    nc.scalar.dma_start(
                        out=outT_ap[:, 512 * b : 512 * b + 256],
                        in_=sb[f"outF{b}"][:, 0:256],
                    )

        # spread items over the early pairs (all before pair npr-2 when
        # possible, so block-3's own k/v items land before they're read).
        spread = max(1, min(npr - 1, 6))
        pipe = []
        for g in range(npr):
            pipe.append(emit_scores(g))
            if g == npr - 1 and stage is not None:
                stage()  # engine-order staged bulk prefetch (scalar queue)
            if g < spread and items:
                budget = (len(items) + (spread - g) - 1) // (spread - g)
                for _ in range(budget):
                    if items:
                        items.pop(0)()
            if len(pipe) > 1:
                emit_outden(pipe.pop(0), last=False)
        while items:  # leftovers (small blocks)
            items.pop(0)()
        while pipe:
            p = pipe.pop(0)
            emit_outden(p, last=not pipe)

        # tail drain: cols [256:512] + this block's denominators
        nc.vector.tensor_copy(sb[f"outF{b}"][:, 256:512], pso[:, 256:])
        if b == NJB - 1:
            nc.sync.dma_start(
                out=outT_ap[:, 512 * b + 256 : 512 * (b + 1)],
                in_=sb[f"outF{b}"][:, 256:512],
            )
        else:
            nc.gpsimd.dma_start(
                out=outT_ap[:, 512 * b : 512 * (b + 1)], in_=sb[f"outF{b}"]
            )
        nc.vector.tensor_copy(sb["denF"][0:1, 512 * b : 512 * (b + 1)], psd)

    # ---- schedule --------------------------------------------------------
    # proj(0) wi-major: q chunk-paced by the arriving DMA, then k/v as
    # one long dense burst (the HAM warm-up trigger, see above).
    for it in proj_items(0):
        it()
    attn(0, proj_items(1),
         stage=lambda: nc.scalar.dma_start(out=sb["xt2"], in_=xt_ap[:, 2]))
    attn(1, proj_items(2),
         stage=lambda: nc.scalar.dma_start(out=sb["xt3"], in_=xt_ap[:, 3]))
    attn(2, proj_items(3, parts=(0, 1)))       # qT/kT of block 3
    attn(3, proj_items(3, parts=(2,)))         # vT + transposes of block 3
    nc.scalar.dma_start(out=den_ap, in_=sb["denF"])


def _build(repeat=1):
    key = ("nc", repeat)
    if key in _CACHE:
        return _CACHE[key]

    import concourse.tile as tile
    from concourse import bacc, mybir

    f32 = mybir.dt.float32
    bf16 = mybir.dt.bfloat16
    fp8 = mybir.dt.float8e4
    nc = bacc.Bacc("TRN2", target_bir_lowering=False, debug=False)

    xt_d = nc.dram_tensor("xt", [128, NJB, EO, 512], bf16, kind="ExternalInput")
    xt8_d = nc.dram_tensor(
        "xt8", [128, len(F8), 3, 2, 512], fp8, kind="ExternalInput"
    )
    w3_d = nc.dram_tensor("w3", [128, EO, 3, H], bf16, kind="ExternalInput")
    w8_d = nc.dram_tensor("w8", [128, 3, 2, 2, H], fp8, kind="ExternalInput")
    consts_d = nc.dram_tensor("consts", [128, 4, 128], bf16, kind="ExternalInput")
    outT_d = nc.dram_tensor("outT", [128, S], bf16, kind="ExternalOutput")
    den_d = nc.dram_tensor("den", [1, S], f32, kind="ExternalOutput")
    dram = (xt_d, xt8_d, w3_d, w8_d, consts_d, outT_d, den_d)

    _CACHE["sb"] = {}
    with tile.TileContext(nc) as tc:
        with (
            tc.tile_pool(name="singles", bufs=1) as singles,
            tc.tile_pool(name="probs", bufs=6) as prb_p,
            tc.tile_pool(name="probs8", bufs=6) as prb8_p,
            tc.tile_pool(name="ps_proj", bufs=2, space="PSUM") as ps_proj,
            tc.tile_pool(name="ps_sc", bufs=2, space="PSUM") as ps_sc,
            tc.tile_pool(name="ps_o", bufs=1, space="PSUM") as ps_o,
            tc.tile_pool(name="ps_d", bufs=1, space="PSUM") as ps_d,
        ):
            pools = (singles, prb_p, prb8_p, ps_proj, ps_sc, ps_o, ps_d)
            for _ in range(repeat):
                _emit_body(nc, tc, pools, dram)

    nc.compile()
    _CACHE[key] = nc
    return nc


def _prep_in_maps(X, padding_mask, W_q, W_k, W_v):
    e4 = ml_dtypes.float8_e4m3
    X = np.asarray(X, dtype=np.float32)
    padding_mask = np.asarray(padding_mask, dtype=np.float32)

    def wprep(W):
        # [H, E] -> [E, H] -> [128(ei), EO, H] with ei innermost of E
        return np.asarray(W, dtype=np.float32).T.reshape(EO, 128, H).transpose(1, 0, 2)

    # [128, EO, 3, H]
    w3 = np.ascontiguousarray(
        np.stack([wprep(W_q), wprep(W_k), wprep(W_v)], axis=2)
    ).astype(ml_dtypes.bfloat16)

    def w8prep(W):
        # [H, E] -> [E, H] -> (3, 2, 128, H) -> [128(ki), 3(c), 2(ko), H]
        a = (np.asarray(W, dtype=np.float32).T * WS).reshape(3, 2, 128, H)
        return a.transpose(2, 0, 1, 3)

    # [128, 3, 2, 2(wi=q,k), H]
    w8 = np.ascontiguousarray(
        np.stack([w8prep(W_q), w8prep(W_k)], axis=3)
    ).astype(e4)

    ident = np.eye(128, dtype=np.float32)
    tri = np.triu(np.ones((128, 128), dtype=np.float32), 1)
    ones = np.ones((128, 128), dtype=np.float32)
    consts = np.ascontiguousarray(
        np.stack(
            [ident, -400.0 * tri, -400.0 * (XS * WS) ** 2 * tri, ones], axis=1
        )
    ).astype(ml_dtypes.bfloat16)  # [128, 4, 128]

    in_maps = []
    for b in range(B):
        Xm = X[b] * padding_mask[b][:, None]  # exact fp32 mask, then quantize
        # [S, E] -> [E, S] -> (3, 2, 128, NJB, 512) -> [128, NJB, 3, 2, 512]
        x8full = (
            (Xm.T * XS)
            .reshape(3, 2, 128, NJB, 512)
            .transpose(2, 3, 0, 1, 4)
        )
        in_maps.append(
            {
                "xt": np.ascontiguousarray(
                    # [S, E] -> [E, S] -> [128(ei), NJB, EO, 512]
                    Xm.T.reshape(EO, 128, NJB, 512).transpose(1, 2, 0, 3)
                ).astype(ml_dtypes.bfloat16),
                "xt8": np.ascontiguousarray(x8full[:, list(F8)]).astype(e4),
                "w3": w3,
                "w8": w8,
                "consts": consts,
            }
        )
    return in_maps


def _finish(res):
    # device wrote outT [128(h), S] bf16 and den [1, S]; out[q, h] = outT.T / den
    return (res["outT"].astype(np.float32).T / res["den"][0][:, None]).astype(
        np.float32
    )


def kernel(X, padding_mask, W_q, W_k, W_v):
    from concourse import bass2jax

    nc = _build(repeat=1)
    in_maps = _prep_in_maps(X, padding_mask, W_q, W_k, W_v)
    results = bass2jax.run_bass_via_pjrt(nc, in_maps, n_cores=B)
    return np.stack([_finish(results[b]) for b in range(B)], axis=0)


# revision 34
# speedup vs baseline: 1.2078x; 1.0249x over previous
"""Causal single-head attention on 8 Trainium2 NeuronCores.

Problem: B=8, S=2048, E=768, HEAD=128, fp32.
  Xm = X * padding_mask[:, :, None]
  q/k/v = Xm @ W_{q,k,v}.T          [B, S, H]
  scores = (q @ k.T) / sqrt(H)  (causal)
  out = softmax(scores) @ v          [B, S, H]

Sharding: pure data-parallel over batch - core b computes batch b; the
tiny projection weights are replicated to every core.

v4 design notes (on top of the bf16 v2 baseline):
  - Startup: inputs stream in ordered globally by first use, with the
    critical sequence (w3[eo01], xt0 in 3 eo-pair chunks, then xt1
    chunks + the fp8 copies) on the sync queue, which serves ~1.5us
    after kernel start at ~250GB/s. Per-chunk completion semaphores let
    each projection matmul wait only on its own chunk. xt2/xt3 bulk is
    issued mid-attention from the scalar engine's program (engine-order
    staging) so it cannot steal startup bandwidth.
  - Projection matmuls for block b+1 are interleaved as small work
    items into the attention pair-pipeline of block b, filling the PE
    bubbles that previously appeared while waiting on ACT exp. Block
    3's projection is split: qT/kT items run inside attn(2), vT +
    transpose items inside attn(3)'s early pairs (legal: pair g only
    touches k-tiles 2g,2g+1, so block-3 k/v tiles are needed only from
    pair 6 on).
  - q/k projections for blocks 2-3 run as fp8(e4m3) DoubleRow matmuls
    (2 contraction elements per cycle, halving their PE time). X and W
    are pre-scaled (x4 / x64) on the host so W escapes the e4m3
    subnormal range; the x65536 score scale folds into the exp scale
    and a second (scaled) causal-mask constant. Host-simulated rel err
    is unchanged (4.8e-3) because the max-error rows live in blocks
    0-1, which stay bf16.
  - Per-block qT/kT/vT/v SBUF tiles so interleaved projection writes
    can never alias attention reads of earlier blocks.
  - outT is drained in bf16 (host divides in fp32); blocks 0-2 go out
    as single 1KB-line transfers on the otherwise-idle gpsimd queue,
    block 3 in two halves on scalar/sync as soon as each half's
    accumulation completes. den goes out as one 8KB DMA at the end.
"""

import math
import sys

import numpy as np

sys.path.insert(0, "/opt/trn_rl_repo")

import ml_dtypes

B, S, E, H = 8, 2048, 768, 128
EO = E // 128          # 6 e-chunks
NJB = S // 512         # 4 q-blocks of 512
SCALE = float(1.0 / math.sqrt(H))
XS, WS = 4.0, 64.0     # fp8 pre-scales for X and W_q/W_k
F8 = (1, 2, 3)         # q-blocks running the fp8 path (block 0 stays bf16)

_CACHE = {}


def _emit_body(nc, tc, pools, dram):
    import concourse.bass as bass  # noqa: F401
    from concourse import mybir

    f32 = mybir.dt.float32
    bf16 = mybir.dt.bfloat16
    fp8 = mybir.dt.float8e4
    DR = mybir.MatmulPerfMode.DoubleRow
    Exp = mybir.ActivationFunctionType.Exp

    singles, prb_p, prb8_p, ps_proj, ps_sc, ps_o, ps_d = pools
    (xt_d, xt8_d, w3_d, w8_d, consts_d, outT_d, den_d) = dram

    sb = _CACHE["sb"]
    if not sb:
        for jb in range(NJB):
            sb[f"xt{jb}"] = singles.tile(
                [128, EO, 512], bf16, tag=f"xt{jb}", name=f"xt{jb}"
            )
            for nm in ("qT", "kT", "vT"):
                sb[f"{nm}{jb}"] = singles.tile(
                    [128, 512], bf16, tag=f"{nm}{jb}", name=f"{nm}{jb}"
                )
            # v in [128, c, t, h] layout so a k-tile pair (for DoubleRow)
            # is the 3D slice [:, c, :, :]
            if jb not in F8:
                sb[f"v{jb}"] = singles.tile(
                    [128, 2, 2, 128], bf16, tag=f"v{jb}", name=f"v{jb}"
                )
            sb[f"v8_{jb}"] = singles.tile(
                [128, 2, 2, 128], fp8, tag=f"v8_{jb}", name=f"v8_{jb}"
            )
            sb[f"outF{jb}"] = singles.tile(
                [128, 512], bf16, tag=f"outF{jb}", name=f"outF{jb}"
            )
        for jb in F8:
            sb[f"xt8_{jb}"] = singles.tile(
                [128, 3, 2, 512], fp8, tag=f"xt8_{jb}", name=f"xt8_{jb}"
            )
        # [128, 2, 16]: the DoubleRow LDW ISA check requires the Ko step
        # to be a multiple of 16 bytes, so the ones column is padded.
        sb["ones8"] = singles.tile([128, 2, 16], fp8, tag="ones8", name="ones8")
        sb["w8"] = singles.tile([128, 3, 2, 2, H], fp8, tag="w8", name="w8")
        sb["w3"] = singles.tile([128, EO, 3, H], bf16, tag="w3", name="w3")
        sb["consts"] = singles.tile([128, 4, 128], bf16, tag="consts", name="consts")
        sb["denF"] = singles.tile([1, S], f32, tag="denF", name="denF")
        sb["warm"] = singles.tile([128, 512], bf16, tag="warm", name="warm")

    xt_ap = xt_d.ap()
    xt8_ap = xt8_d.ap()
    w3_ap = w3_d.ap()
    outT_ap = outT_d.ap()
    den_ap = den_d.ap()

    # ---- prologue loads --------------------------------------------------
    # sync queue: the startup-critical sequence, ordered by first use.
    # scalar joins ~2us later with the rest of w3 + consts; its bulk
    # (xt2/xt3) is staged from inside attn(0)/attn(1) instead. gpsimd
    # stays free for output drains.
    nc.gpsimd.memset(sb["warm"], 0.125)
    nc.gpsimd.memset(sb["ones8"], 1.0)
    nc.sync.dma_start(out=sb["w3"][:, 0:2], in_=w3_ap[:, 0:2])
    for p in range(2):
        nc.sync.dma_start(
            out=sb["xt0"][:, 2 * p : 2 * p + 2], in_=xt_ap[:, 0, 2 * p : 2 * p + 2]
        )
    for p in range(2):
        nc.sync.dma_start(
            out=sb["xt1"][:, 2 * p : 2 * p + 2], in_=xt_ap[:, 1, 2 * p : 2 * p + 2]
        )
    nc.sync.dma_start(out=sb["w8"], in_=w8_d.ap())
    for i, jb in enumerate(F8):
        nc.sync.dma_start(out=sb[f"xt8_{jb}"], in_=xt8_ap[:, i])
    # the LAST eo-pair chunk of xt0 and xt1 rides the scalar queue (idle
    # after w3/consts) so the startup-critical bytes stream on two queues
    # in parallel - covers the runs where the sync queue serves slowly.
    nc.scalar.dma_start(out=sb["w3"][:, 2:4], in_=w3_ap[:, 2:4])
    nc.scalar.dma_start(out=sb["w3"][:, 4:6], in_=w3_ap[:, 4:6])
    nc.scalar.dma_start(out=sb["consts"], in_=consts_d.ap())
    nc.scalar.dma_start(out=sb["xt0"][:, 4:6], in_=xt_ap[:, 0, 4:6])
    nc.scalar.dma_start(out=sb["xt1"][:, 4:6], in_=xt_ap[:, 1, 4:6])

    ident = sb["consts"][:, 0, :]
    triA = sb["consts"][:, 1, :]       # -400 (bf16-scale blocks)
    triA8 = sb["consts"][:, 2, :]      # -400 * (XS*WS)^2 (fp8-scale blocks)
    ones1 = sb["consts"][:, 3, 0:1]

    # Short PE warmup bridging the gap until the first chunks land. It
    # deliberately does NOT try to flip the HAM clock-gate early: during
    # the chunk-paced q projection the PE is cold (longer matmuls = a
    # higher busy-fraction across DMA waits), and the dense 12-matmul
    # k/v burst right after the last chunk flips HAM exactly once, with
    # no re-throttle. Early-warm variants lose ~4us to HAM oscillation
    # whenever the startup DMA runs slow.
    ps_warm = ps_proj.tile([128, 512], f32, tag="proj", name="ps_warm")
    for _ in range(5):
        nc.tensor.matmul(
            ps_warm, lhsT=sb["warm"][:, 0:128], rhs=sb["warm"], start=True, stop=True
        )

    # ---- projection work items ------------------------------------------
    # proj(b) = 11 small PE items: 3 per weight (chunk-paced; the last
    # one adds the PSUM->SBUF copy) + 2 transpose items for v. For F8
    # blocks the q/k items are single fp8 DoubleRow matmuls over an
    # eo-pair (256-wide contraction at 2 elems/cycle).
    def proj_items(b, parts=(0, 1, 2), chunk_major=False):
        items = []
        cell = {}
        use8 = b in F8
        x8 = sb[f"xt8_{b}"] if use8 else None

        def mm_item(wi, p, nm, b=b):
            def run():
                if p == 0:
                    cell[wi] = ps_proj.tile(
                        [128, 512], f32, tag="proj", name=f"ps_{nm}{b}"
                    )
                ps = cell[wi]
                if use8 and wi < 2:
                    nc.tensor.matmul(
                        ps,
                        lhsT=sb["w8"][:, p, :, wi, :],
                        rhs=x8[:, p, :, :],
                        start=(p == 0),
                        stop=(p == 2),
                        perf_mode=DR,
                    )
                else:
                    for eo in (2 * p, 2 * p + 1):
                        nc.tensor.matmul(
                            ps,
                            lhsT=sb["w3"][:, eo, wi, :],
                            rhs=sb[f"xt{b}"][:, eo, :],
                            start=(eo == 0),
                            stop=(eo == EO - 1),
                        )
                if p == 2:
                    nc.vector.tensor_copy(sb[f"{nm}{b}"], ps)

            return run

        def tr_item(half, b=b):
            def run():
                if half == 0:
                    cell["psv"] = ps_proj.tile(
                        [128, 2, 2, 128], bf16, tag="proj", name=f"psv{b}"
                    )
                psv = cell["psv"]
                for c in (2 * half, 2 * half + 1):
                    nc.tensor.transpose(
                        psv[:, c // 2, c % 2, :],
                        sb[f"vT{b}"][:, 128 * c : 128 * (c + 1)],
                        ident,
                    )
                if half == 1:
                    # fp8 copy always (attn(2)/(3) read every block's v in
                    # fp8); bf16 copy only for blocks attn(0)/(1) touch.
                    nc.vector.tensor_copy(sb[f"v8_{b}"], psv)
                    if b not in F8:
                        nc.vector.tensor_copy(sb[f"v{b}"], psv)

            return run

        if chunk_major:
            # q/k rows interleaved per eo-chunk: 4 matmuls of ready work per
            # chunk arrival, so the PE never idles (and HAM never
            # re-throttles) while the startup chunks stream in.
            for p in range(3):
                items.append(mm_item(0, p, "qT"))
                items.append(mm_item(1, p, "kT"))
            for p in range(3):
                items.append(mm_item(2, p, "vT"))
            items.append(tr_item(0))
            items.append(tr_item(1))
            return items
        worder = ((2, "vT"), (0, "qT"), (1, "kT")) if use8 else (
            (0, "qT"), (1, "kT"), (2, "vT"))
        for wi, nm in worder:
            if wi not in parts:
                continue
            for p in range(3):
                items.append(mm_item(wi, p, nm))
            if wi == 2:
                items.append(tr_item(0))
                items.append(tr_item(1))
        return items

    # ---- attention for q-block b, interleaving `items` into the pairs ----
    def attn(b, items, stage=None):
        nkt = 4 * (b + 1)          # causal: k tiles 0 .. 4b+3
        npr = nkt // 2
        pso = ps_o.tile([128, 512], f32, tag="o", name=f"pso_{b}")
        psd = ps_d.tile([1, 512], f32, tag="d", name=f"psd_{b}")
        qT = sb[f"qT{b}"]
        sq = 1.0 / (XS * WS) if b in F8 else 1.0

        def off_of(i):
            return 128 * (i - 4 * b) if i >= 4 * b else 0

        use8 = b in F8            # fp8 probs/v + DoubleRow out/den
        vpfx = "v8_" if use8 else "v"

        def kt_of(i):
            return sb[f"kT{i // 4}"][:, 128 * (i % 4) : 128 * (i % 4 + 1)]

        def v_of(i):
            return sb[f"{vpfx}{i // 4}"][:, (i % 4) // 2, (i % 4) % 2, :]

        def emit_scores(g):
            kb = (2 * g) // 4      # pairs never straddle a k-block boundary
            sk = 1.0 / (XS * WS) if kb in F8 else 1.0
            pssc = ps_sc.tile([128, 2, 512], f32, tag="sc", name=f"sc_{b}_{g}")
            for t in range(2):
                i = 2 * g + t
                diag = i >= 4 * b
                off = off_of(i)
                nc.tensor.matmul(
                    pssc[:, t, off:],
                    lhsT=kt_of(i),
                    rhs=qT[:, off:],
                    start=True,
                    stop=not diag,
                )
                if diag:  # add -400*(scale) strictly-upper triangle pre-exp
                    nc.tensor.matmul(
                        pssc[:, t, off : off + 128],
                        lhsT=(triA8 if b in F8 else triA),
                        rhs=ident,
                        start=False,
                        stop=True,
                    )
            moff = off_of(2 * g)
            if use8:
                prb = prb8_p.tile([128, 2, 512], fp8, tag="pr8", name=f"prb_{b}_{g}")
            else:
                prb = prb_p.tile([128, 2, 512], bf16, tag="pr", name=f"prb_{b}_{g}")
            nc.scalar.activation(
                prb[:, :, moff:], pssc[:, :, moff:], Exp, scale=SCALE * sq * sk
            )
            return (g, prb)

        def emit_outden(pend, last):
            g, pprb = pend
            if use8 and 2 * g + 1 < 4 * b:
                # non-diag pair, fp8: one DoubleRow matmul contracts both
                # k-tiles of the pair at 2 elements/cycle (for out and den)
                kb, c = (2 * g) // 4, ((2 * g) % 4) // 2
                nc.tensor.matmul(
                    pso,
                    lhsT=sb[f"v8_{kb}"][:, c, :, :],
                    rhs=pprb,
                    start=(g == 0),
                    stop=False,
                    perf_mode=DR,
                )
                nc.tensor.matmul(
                    psd,
                    lhsT=sb["ones8"][:, :, 0:1],
                    rhs=pprb,
                    start=(g == 0),
                    stop=False,
                    perf_mode=DR,
                )
            else:
                dlhs = sb["ones8"][:, 0, 0:1] if use8 else ones1
                for t in range(2):
                    i = 2 * g + t
                    off = off_of(i)
                    nc.tensor.matmul(
                        pso[:, off:],
                        lhsT=v_of(i),
                        rhs=pprb[:, t, off:],
                        start=(i == 0),
                        stop=last and t == 1,
                    )
                for t in range(2):
                    i = 2 * g + t
                    off = off_of(i)
                    nc.tensor.matmul(
                        psd[:, off:],
                        lhsT=dlhs,
                        rhs=pprb[:, t, off:],
                        start=(i == 0),
                        stop=last and t == 1,
                    )
            # cols [0:256] final once the off=128 diag tile has run: drain
            # early so the tail copy+DMA overlaps the last pair. Only the
            # last block DMAs its halves separately (tail latency); the
            # others go out as one 1KB-line transfer on the idle gpsimd
            # queue after the tail copy.
            if g == npr - 2:
                nc.vector.tensor_copy(sb[f"outF{b}"][:, 0:256], pso[:, 0:256])
                if b == NJB - 1:
                    nc.scalar.dma_start(
                        out=outT_ap[:, 512 * b : 512 * b + 256],
                        in_=sb[f"outF{b}"][:, 0:256],
                    )

        # spread items over the early pairs (all before pair npr-2 when
        # possible, so block-3's own k/v items land before they're read).
        spread = max(1, min(npr - 1, 6))
        pipe = []
        for g in range(npr):
            pipe.append(emit_scores(g))
            if g == npr - 1 and stage is not None:
                stage()  # engine-order staged bulk prefetch (scalar queue)
            if g < spread and items:
                budget = (len(items) + (spread - g) - 1) // (spread - g)
                for _ in range(budget):
                    if items:
                        items.pop(0)()
            if len(pipe) > 1:
                emit_outden(pipe.pop(0), last=False)
        while items:  # leftovers (small blocks)
            items.pop(0)()
        while pipe:
            p = pipe.pop(0)
            emit_outden(p, last=not pipe)

        # tail drain: cols [256:512] + this block's denominators
        nc.vector.tensor_copy(sb[f"outF{b}"][:, 256:512], pso[:, 256:])
        if b == NJB - 1:
            nc.sync.dma_start(
                out=outT_ap[:, 512 * b + 256 : 512 * (b + 1)],
                in_=sb[f"outF{b}"][:, 256:512],
            )
        else:
            nc.gpsimd.dma_start(
                out=outT_ap[:, 512 * b : 512 * (b + 1)], in_=sb[f"outF{b}"]
            )
        nc.vector.tensor_copy(sb["denF"][0:1, 512 * b : 512 * (b + 1)], psd)

    # ---- schedule --------------------------------------------------------
    # proj(0) wi-major: q chunk-paced by the arriving DMA, then k/v as
    # one long dense burst (the HAM warm-up trigger, see above).
    for it in proj_items(0):
        it()
    attn(0, proj_items(1),
         stage=lambda: nc.scalar.dma_start(out=sb["xt2"], in_=xt_ap[:, 2]))
    attn(1, proj_items(2),
         stage=lambda: nc.scalar.dma_start(out=sb["xt3"], in_=xt_ap[:, 3]))
    attn(2, proj_items(3, parts=(0, 1)))       # qT/kT of block 3
    attn(3, proj_items(3, parts=(2,)))         # vT + transposes of block 3
    nc.scalar.dma_start(out=den_ap, in_=sb["denF"])


def _build(repeat=1):
    key = ("nc", repeat)
    if key in _CACHE:
        return _CACHE[key]

    import concourse.tile as tile
    from concourse import bacc, mybir

    f32 = mybir.dt.float32
    bf16 = mybir.dt.bfloat16
    fp8 = mybir.dt.float8e4
    nc = bacc.Bacc("TRN2", target_bir_lowering=False, debug=False)

    xt_d = nc.dram_tensor("xt", [128, NJB, EO, 512], bf16, kind="ExternalInput")
    xt8_d = nc.dram_tensor(
        "xt8", [128, len(F8), 3, 2, 512], fp8, kind="ExternalInput"
    )
    w3_d = nc.dram_tensor("w3", [128, EO, 3, H], bf16, kind="ExternalInput")
    w8_d = nc.dram_tensor("w8", [128, 3, 2, 2, H], fp8, kind="ExternalInput")
    consts_d = nc.dram_tensor("consts", [128, 4, 128], bf16, kind="ExternalInput")
    outT_d = nc.dram_tensor("outT", [128, S], bf16, kind="ExternalOutput")
    den_d = nc.dram_tensor("den", [1, S], f32, kind="ExternalOutput")
    dram = (xt_d, xt8_d, w3_d, w8_d, consts_d, outT_d, den_d)

    _CACHE["sb"] = {}
    with tile.TileContext(nc) as tc:
        with (
            tc.tile_pool(name="singles", bufs=1) as singles,
            tc.tile_pool(name="probs", bufs=6) as prb_p,
            tc.tile_pool(name="probs8", bufs=6) as prb8_p,
            tc.tile_pool(name="ps_proj", bufs=2, space="PSUM") as ps_proj,
            tc.tile_pool(name="ps_sc", bufs=2, space="PSUM") as ps_sc,
            tc.tile_pool(name="ps_o", bufs=1, space="PSUM") as ps_o,
            tc.tile_pool(name="ps_d", bufs=1, space="PSUM") as ps_d,
        ):
            pools = (singles, prb_p, prb8_p, ps_proj, ps_sc, ps_o, ps_d)
            for _ in range(repeat):
                _emit_body(nc, tc, pools, dram)

    nc.compile()
    _CACHE[key] = nc
    return nc


def _prep_in_maps(X, padding_mask, W_q, W_k, W_v):
    e4 = ml_dtypes.float8_e4m3
    X = np.asarray(X, dtype=np.float32)
    padding_mask = np.asarray(padding_mask, dtype=np.float32)

    def wprep(W):
        # [H, E] -> [E, H] -> [128(ei), EO, H] with ei innermost of E
        return np.asarray(W, dtype=np.float32).T.reshape(EO, 128, H).transpose(1, 0, 2)

    # [128, EO, 3, H]
    w3 = np.ascontiguousarray(
        np.stack([wprep(W_q), wprep(W_k), wprep(W_v)], axis=2)
    ).astype(ml_dtypes.bfloat16)

    def w8prep(W):
        # [H, E] -> [E, H] -> (3, 2, 128, H) -> [128(ki), 3(c), 2(ko), H]
        a = (np.asarray(W, dtype=np.float32).T * WS).reshape(3, 2, 128, H)
        return a.transpose(2, 0, 1, 3)

    # [128, 3, 2, 2(wi=q,k), H]
    w8 = np.ascontiguousarray(
        np.stack([w8prep(W_q), w8prep(W_k)], axis=3)
    ).astype(e4)

    ident = np.eye(128, dtype=np.float32)
    tri = np.triu(np.ones((128, 128), dtype=np.float32), 1)
    ones = np.ones((128, 128), dtype=np.float32)
    consts = np.ascontiguousarray(
        np.stack(
            [ident, -400.0 * tri, -400.0 * (XS * WS) ** 2 * tri, ones], axis=1
        )
    ).astype(ml_dtypes.bfloat16)  # [128, 4, 128]

    in_maps = []
    for b in range(B):
        Xm = X[b] * padding_mask[b][:, None]  # exact fp32 mask, then quantize
        # [S, E] -> [E, S] -> (3, 2, 128, NJB, 512) -> [128, NJB, 3, 2, 512]
        x8full = (
            (Xm.T * XS)
            .reshape(3, 2, 128, NJB, 512)
            .transpose(2, 3, 0, 1, 4)
        )
        in_maps.append(
            {
                "xt": np.ascontiguousarray(
                    # [S, E] -> [E, S] -> [128(ei), NJB, EO, 512]
                    Xm.T.reshape(EO, 128, NJB, 512).transpose(1, 2, 0, 3)
                ).astype(ml_dtypes.bfloat16),
                "xt8": np.ascontiguousarray(x8full[:, list(F8)]).astype(e4),
                "w3": w3,
                "w8": w8,
                "consts": consts,
            }
        )
    return in_maps


def _finish(res):
    # device wrote outT [128(h), S] bf16 and den [1, S]; out[q, h] = outT.T / den
    return (res["outT"].astype(np.float32).T / res["den"][0][:, None]).astype(
        np.float32
    )


def kernel(X, padding_mask, W_q, W_k, W_v):
    from concourse import bass2jax

    nc = _build(repeat=1)
    in_maps = _prep_in_maps(X, padding_mask, W_q, W_k, W_v)
    results = bass2jax.run_bass_via_pjrt(nc, in_maps, n_cores=B)
    return np.stack([_finish(results[b]) for b in range(B)], axis=0)
